# revision 14
# baseline (speedup 1.0000x reference)
"""Trainium2 Bass kernel for nn_Decoder: out = (x - b_pre) @ W^T.

Shapes (hardcoded): x [8192, 32768] f32, W [768, 32768] f32, b_pre [32768] f32
-> out [8192, 768] f32.

Sharding: data-parallel over the 8192 token rows across 8 NeuronCores
(1024 rows each), W replicated. The TensorE contracts over the partition
axis, so both operands are fed with the contraction dim (d = 32768) on
partitions: the host pre-transposes each x shard to xT [32768, 1024] and
W to wT [32768, 768] (cheap, ~2 s total). b_pre is folded into x on the
host (x - b_pre) before the transpose; with the reference's b_pre == 0
this is bitwise a no-op.

Default per-core kernel (DESIGN="sbuf", MM_DTYPE="float32r"): stream d
in 32 supers of 8x128 rows; each super DMAs 8 xT chunks [128, 1024] and
8 wT chunks [128, 768] (both tensors touch HBM exactly once, 227 MiB).
For each of 8 output row-chunks, 16 matmuls accumulate the super's
contraction into a [128, 768] PSUM tile (x chunk as the self-loading
stationary operand, wT as the 512/256-col moving operand), and the DVE
adds the PSUM tile into the SBUF-resident [1024, 768] output. x DMAs
issue from the SP HWDGE engine and W DMAs from ACT, halving per-engine
descriptor-issue load. Measured: 758 us HW at 95.7% PE-matmul
occupancy; float32r streams moving columns at ~9/8 cyc/col, so the PE
floor is 737 us and the structural floor (plus ~24 us fixed Tile
preamble/drain) is ~760 us. Scale-relative error 1.6e-4 vs fp64.
"float32" mode is exact (1e-6) at ~2.66 ms.

Tuning notes: DSUP=16 regresses (prefetch margin too thin -> PE input
waits + cold clock); XT/WT_BUFS=22 overflows SBUF; single-engine DMA
issue costs ~9 us; 16/16/3 + dual-engine issue is the optimum found.
"""

import os
import sys

if "/opt/trn_rl_repo" not in sys.path:
    sys.path.insert(0, "/opt/trn_rl_repo")

import numpy as np


def _shim_axon_hooks():
    """The agent image's `antenv` package lacks `axon_hooks`; when a
    caller sets BASS_TRACE=1 the bass trace path would ImportError. Shim
    the module (registering the same ctypes NTFF hook trn_boot would
    have) so tracing degrades gracefully instead of crashing."""
    try:
        import antenv.axon_hooks  # noqa: F401
        return
    except ImportError:
        pass
    try:
        import types

        import antenv
        from trn_agent_boot.trn_boot import _ntff_profile_via_ctypes

        m = types.ModuleType("antenv.axon_hooks")
        m._hook = _ntff_profile_via_ctypes("/opt/axon/libaxon_pjrt.so")
        m.get_axon_ntff_profile_hook = lambda: m._hook
        m.set_axon_ntff_profile_hook = lambda h: setattr(m, "_hook", h)
        sys.modules["antenv.axon_hooks"] = m
        antenv.axon_hooks = m
    except Exception:
        pass


_shim_axon_hooks()

N_TOK = 8192
D_IN = 32768
D_OUT = 768
N_CORES = 8
N_SHARD = N_TOK // N_CORES          # 1024 token rows per core
P = 128
D_CHUNKS = D_IN // P                # 256
N_SUPER = 512                       # token rows resident in PSUM at once
N_SUPERS = N_SHARD // N_SUPER       # 2
N_CH = N_SUPER // P                 # 4 psum tiles per n-block

# Matmul input dtype knob:
#   "bfloat16": 1 cyc/col on the PE (78.6 TF/s/core) and half the HBM
#       traffic of fp32. Scale-relative error ~1.5e-3 at K=32768 --
#       well under the 2e-2 gate.
#   "float32r": single-pass fp32 PE matmul, ~9/8 cyc/col measured, max
#       scale-relative error 1.6e-4.
#   "float32": exact to 1e-6 but 4 cyc/col -> ~3.5x slower.
MM_DTYPE = os.environ.get("KERNEL_MM_DTYPE", "bfloat16")

# Max acceptable scale-relative error (vs fp64, 64 sampled rows) before
# kernel() falls back to a more exact dtype.
_ERR_GATE = {"bfloat16": 8e-3, "float16": 8e-3, "float32r": 5e-3,
             "float32": 1e-4}
# "sbuf": d-super blocking, output accumulated in SBUF, min DMA traffic
#         (best: 766 us HW).
# "psum": full-K accumulation in PSUM, W streamed twice (simplest).
# "kshard"/"kshard_ot": tensor-parallel over the contraction dim.
DESIGN = os.environ.get("KERNEL_DESIGN", "sbuf")

LAST_RESULTS = None  # BassKernelResults of the most recent kernel() call


W8_SCALE = 128.0  # host pre-scales the fp8 W shard into e4m3 normal range


def _f8_pairs() -> int:
    """Chunk-PAIRS (2x128 K rows each) computed in fp8 DoubleRow. Only
    meaningful for the bfloat16 fast path."""
    if MM_DTYPE != "bfloat16":
        return 0
    return _F8_STATE[0]


_F8_STATE = [int(os.environ.get("KERNEL_F8_PAIRS", "12"))]


def _build_bass_sbuf():
    """Design 1: stream xT and wT exactly once in d-supers; accumulate
    the [1024, 768] output in SBUF across d-supers (DVE adds PSUM into
    the resident C tiles).

    Refinements over the fp32r baseline:
    - warmup ramp: the first two supers are half-size (4 chunks) so the
      first matmuls don't outrun the just-started DMA stream;
    - optional fp8 tail: the last 2*F8 d-chunks are computed in
      DoubleRow fp8e4 (2x PE rate, K=256 per matmul). W is pre-scaled
      by W8_SCALE on the host; the drain folds 1/W8_SCALE back in via
      scalar_tensor_tensor (ct = ps*(1/s) + ct);
    - per-chunk output DMAs issued right after each chunk's last drain,
      alternating the sync/scalar issue queues, so the writeback
      overlaps the remaining compute."""
    import concourse.mybir as mybir
    import concourse.tile as tile
    from concourse import bacc

    dt_mm = getattr(mybir.dt, MM_DTYPE)
    f32 = mybir.dt.float32
    f8 = mybir.dt.float8e4
    F8 = _f8_pairs()
    D8 = 2 * F8 * P                # fp8 K rows (tail of the d axis)
    DB = D_IN - D8                 # bf16/fp32 K rows
    CHB = DB // P                  # full-precision d-chunks
    DSUP = int(os.environ.get("KERNEL_DSUP", "8"))  # d-chunks per super
    sched = [DSUP] * (CHB // DSUP)
    assert sum(sched) == CHB, (sched, CHB)
    NCH = N_SHARD // P             # 8 output row-chunks

    nc = bacc.Bacc(None, target_bir_lowering=False)
    xT = nc.dram_tensor("xT", [DB, N_SHARD], dt_mm, kind="ExternalInput")
    wT = nc.dram_tensor("wT", [DB, D_OUT], dt_mm, kind="ExternalInput")
    if F8:
        x8T = nc.dram_tensor("x8T", [D8, N_SHARD], f8, kind="ExternalInput")
        w8T = nc.dram_tensor("w8T", [D8, D_OUT], f8, kind="ExternalInput")
    out = nc.dram_tensor("out", [N_SHARD, D_OUT], f32, kind="ExternalOutput")

    XT_BUFS = int(os.environ.get("KERNEL_XT_BUFS", "16"))
    WT_BUFS = int(os.environ.get("KERNEL_WT_BUFS", "16"))
    # DoubleRow-mode psum tiles are charged 2x banks by the allocator,
    # so with the fp8 tail only 2 bufs fit (2x4 banks); still enough to
    # hide each drain under the next matmul group.
    PS_BUFS = int(os.environ.get("KERNEL_PS_BUFS", "2" if F8 else "3"))
    mul = mybir.AluOpType.mult
    add = mybir.AluOpType.add
    with tile.TileContext(nc) as tc:
        with (
            tc.tile_pool(name="xs", bufs=XT_BUFS) as xpool,
            tc.tile_pool(name="ws", bufs=WT_BUFS) as wpool,
            tc.tile_pool(name="c", bufs=1) as cpool,
            tc.tile_pool(name="f8x", bufs=max(F8, 1)) as f8xpool,
            tc.tile_pool(name="f8w", bufs=max(F8, 1)) as f8wpool,
            tc.tile_pool(name="psum", bufs=PS_BUFS, space="PSUM") as ppool,
        ):
            cts = [
                cpool.tile([P, D_OUT], f32, name=f"c{i}") for i in range(NCH)
            ]
            x8ts = []
            w8ts = []
            chunk0 = 0
            for si, DS in enumerate(sched):
                # Per-chunk tiles (not one slab) so the first matmul of a
                # super only waits on one chunk DMA, and prefetch runs
                # chunk-granular across supers.
                xts = []
                wts = []
                for j in range(DS):
                    row = (chunk0 + j) * P
                    xt = xpool.tile([P, N_SHARD], dt_mm, name="xt")
                    wt = wpool.tile([P, D_OUT], dt_mm, name="wt")
                    # Split descriptor issue across the two HWDGE engines
                    # (SP + ACT) so x and W prefetch don't queue behind
                    # each other on one issue path.
                    nc.sync.dma_start(xt[:], xT[row:row + P, :])
                    nc.scalar.dma_start(wt[:], wT[row:row + P, :])
                    xts.append(xt)
                    wts.append(wt)
                if si == len(sched) - 1 and F8:
                    # fp8 tail operands: issued after all bf16 prefetch
                    # DMAs so they never starve the main stream; they
                    # land well before the fp8 section runs.
                    for p8 in range(F8):
                        x8t = f8xpool.tile([P, 2, N_SHARD], f8, name="x8t")
                        w8t = f8wpool.tile([P, 2, D_OUT], f8, name="w8t")
                        for ks in range(2):
                            row = (2 * p8 + ks) * P
                            nc.sync.dma_start(x8t[:, ks, :],
                                              x8T[row:row + P, :])
                            nc.scalar.dma_start(w8t[:, ks, :],
                                                w8T[row:row + P, :])
                        x8ts.append(x8t)
                        w8ts.append(w8t)
                for nch in range(NCH):
                    ps = ppool.tile([P, D_OUT], f32, name="ps")
                    for j in range(DS):
                        lhsT = xts[j][:, nch * P:(nch + 1) * P]
                        nc.tensor.matmul(
                            ps[:, 0:512], lhsT, wts[j][:, 0:512],
                            start=(j == 0), stop=(j == DS - 1),
                        )
                        nc.tensor.matmul(
                            ps[:, 512:D_OUT], lhsT, wts[j][:, 512:D_OUT],
                            start=(j == 0), stop=(j == DS - 1),
                        )
                    if si == 0:
                        nc.vector.tensor_copy(cts[nch][:], ps[:])
                    else:
                        nc.vector.tensor_add(cts[nch][:], cts[nch][:], ps[:])
                    if si == len(sched) - 1 and not F8:
                        eng = nc.sync if nch % 2 == 0 else nc.scalar
                        eng.dma_start(out[nch * P:(nch + 1) * P, :],
                                      cts[nch][:])
                chunk0 += DS
            if F8:
                dr = mybir.MatmulPerfMode.DoubleRow
                for nch in range(NCH):
                    ps = ppool.tile([P, D_OUT], f32, name="ps8")
                    for p8 in range(F8):
                        lhsT = x8ts[p8][:, :, nch * P:(nch + 1) * P]
                        nc.tensor.matmul(
                            ps[:, 0:512], lhsT, w8ts[p8][:, :, 0:512],
                            start=(p8 == 0), stop=(p8 == F8 - 1),
                            perf_mode=dr,
                        )
                        nc.tensor.matmul(
                            ps[:, 512:D_OUT], lhsT,
                            w8ts[p8][:, :, 512:D_OUT],
                            start=(p8 == 0), stop=(p8 == F8 - 1),
                            perf_mode=dr,
                        )
                    nc.vector.scalar_tensor_tensor(
                        cts[nch][:], ps[:], 1.0 / W8_SCALE, cts[nch][:],
                        op0=mul, op1=add,
                    )
                    eng = nc.sync if nch % 2 == 0 else nc.scalar
                    eng.dma_start(out[nch * P:(nch + 1) * P, :], cts[nch][:])

    nc.compile()
    return nc


def _build_bass_dp_ot():
    """Design 5 (data-parallel, W-stationary, all-512 moving): like
    "sbuf" but with wT chunks as the stationary operand and xT as the
    512-wide moving operand, so every matmul streams 512 cols (213 ns at
    bf16) and fully hides its ~97 ns LDWEIGHTS (the x-stationary 256-col
    matmuls at 107 ns barely cover it). Output accumulates in SBUF
    transposed [768, N_SHARD]; the host transposes back at gather. Final
    out DMAs alternate sync/scalar queues and are issued per-chunk so the
    drain overlaps the last super's adds."""
    import concourse.mybir as mybir
    import concourse.tile as tile
    from concourse import bacc

    dt_mm = getattr(mybir.dt, MM_DTYPE)
    f32 = mybir.dt.float32
    DSUP = int(os.environ.get("KERNEL_DSUP", "8"))  # d-chunks per super
    NSUP = D_CHUNKS // DSUP        # supers
    OC = D_OUT // P                # 6 output-channel chunks
    TH = N_SHARD // 512            # 2 moving halves of 512 tokens

    nc = bacc.Bacc(None, target_bir_lowering=False)
    xT = nc.dram_tensor("xT", [D_IN, N_SHARD], dt_mm, kind="ExternalInput")
    wT = nc.dram_tensor("wT", [D_IN, D_OUT], dt_mm, kind="ExternalInput")
    outT = nc.dram_tensor("outT", [D_OUT, N_SHARD], f32, kind="ExternalOutput")

    XT_BUFS = int(os.environ.get("KERNEL_XT_BUFS", "16"))
    WT_BUFS = int(os.environ.get("KERNEL_WT_BUFS", "16"))
    PS_BUFS = int(os.environ.get("KERNEL_PS_BUFS", "3"))
    with tile.TileContext(nc) as tc:
        with (
            tc.tile_pool(name="xs", bufs=XT_BUFS) as xpool,
            tc.tile_pool(name="ws", bufs=WT_BUFS) as wpool,
            tc.tile_pool(name="c", bufs=1) as cpool,
            tc.tile_pool(name="psum", bufs=PS_BUFS, space="PSUM") as ppool,
        ):
            cts = [
                cpool.tile([P, N_SHARD], f32, name=f"c{i}") for i in range(OC)
            ]
            for ds in range(NSUP):
                xts = []
                wts = []
                for j in range(DSUP):
                    row = (ds * DSUP + j) * P
                    xt = xpool.tile([P, N_SHARD], dt_mm, name="xt")
                    wt = wpool.tile([P, D_OUT], dt_mm, name="wt")
                    nc.sync.dma_start(xt[:], xT[row:row + P, :])
                    nc.scalar.dma_start(wt[:], wT[row:row + P, :])
                    xts.append(xt)
                    wts.append(wt)
                for oc in range(OC):
                    for th in range(TH):
                        ps = ppool.tile([P, 512], f32, name="ps")
                        for j in range(DSUP):
                            nc.tensor.matmul(
                                ps[:],
                                wts[j][:, oc * P:(oc + 1) * P],
                                xts[j][:, th * 512:(th + 1) * 512],
                                start=(j == 0), stop=(j == DSUP - 1),
                            )
                        dst = cts[oc][:, th * 512:(th + 1) * 512]
                        if ds == 0:
                            nc.vector.tensor_copy(dst, ps[:])
                        else:
                            nc.vector.tensor_add(dst, dst, ps[:])
                    if ds == NSUP - 1:
                        eng = nc.sync if oc % 2 == 0 else nc.scalar
                        eng.dma_start(outT[oc * P:(oc + 1) * P, :], cts[oc][:])

    nc.compile()
    return nc


def _build_bass_kshard():
    """Design 3 (tensor-parallel): shard the contraction dim d across
    cores (4096 rows each). The W^T shard [4096, 768] (12 MiB) stays
    resident in SBUF; x^T [4096, 8192] streams through once. Each core
    produces a full [8192, 768] partial; the host reduces the 8 partials
    at gather time (the sharding hint's "all-reduce on the [N,768]
    output"). PSUM accumulates the core's entire local contraction."""
    import concourse.mybir as mybir
    import concourse.tile as tile
    from concourse import bacc

    dt_mm = getattr(mybir.dt, MM_DTYPE)
    f32 = mybir.dt.float32
    D_SHARD = D_IN // N_CORES       # 4096 contraction rows per core
    DC = D_SHARD // P               # 32 d-chunks
    NB = N_TOK // N_SUPER           # 16 n-blocks of 512 token rows

    nc = bacc.Bacc(None, target_bir_lowering=False)
    xT = nc.dram_tensor("xT", [D_SHARD, N_TOK], dt_mm, kind="ExternalInput")
    wT = nc.dram_tensor("wT", [D_SHARD, D_OUT], dt_mm, kind="ExternalInput")
    out = nc.dram_tensor("out", [N_TOK, D_OUT], f32, kind="ExternalOutput")

    with tile.TileContext(nc) as tc:
        with (
            tc.tile_pool(name="w", bufs=1) as wpool,
            tc.tile_pool(name="xt", bufs=4) as xpool,
            tc.tile_pool(name="ot", bufs=4) as opool,
            tc.tile_pool(name="psum", bufs=1, space="PSUM") as ppool,
        ):
            ws = wpool.tile([P, DC, D_OUT], dt_mm, name="ws")
            for j in range(DC):
                nc.sync.dma_start(ws[:, j, :], wT[j * P:(j + 1) * P, :])
            for nb in range(NB):
                psums = [
                    ppool.tile([P, D_OUT], f32, name=f"psum{i}")
                    for i in range(N_CH)
                ]
                for dc in range(DC):
                    xt = xpool.tile([P, N_SUPER], dt_mm, name="xt")
                    nc.sync.dma_start(
                        xt[:],
                        xT[dc * P:(dc + 1) * P,
                           nb * N_SUPER:(nb + 1) * N_SUPER],
                    )
                    st = dc == 0
                    sp = dc == DC - 1
                    for nch in range(N_CH):
                        lhsT = xt[:, nch * P:(nch + 1) * P]
                        nc.tensor.matmul(
                            psums[nch][:, 0:512], lhsT, ws[:, dc, 0:512],
                            start=st, stop=sp,
                        )
                        nc.tensor.matmul(
                            psums[nch][:, 512:D_OUT], lhsT,
                            ws[:, dc, 512:D_OUT],
                            start=st, stop=sp,
                        )
                for nch in range(N_CH):
                    ot = opool.tile([P, D_OUT], f32, name="ot")
                    nc.vector.tensor_copy(ot[:], psums[nch][:])
                    base = nb * N_SUPER + nch * P
                    nc.sync.dma_start(out[base:base + P, :], ot[:])

    nc.compile()
    return nc


def _build_bass_kshard_ot():
    """Design 4 (tensor-parallel, W-stationary): like kshard, but W^T
    tiles are the stationary operand and x^T streams as the moving side,
    so every matmul has a 512-wide moving operand. For float32r each
    matmul self-loads its stationary via a ~214 ns LDWEIGHTS; with all
    matmuls at N=512 (213 ns) the loads pipeline behind the previous
    matmul instead of stalling (the N=256 matmuls of the x-stationary
    designs are LDW-bound). Output lands transposed [768, 8192]; the
    host transposes back during the reduce."""
    import concourse.mybir as mybir
    import concourse.tile as tile
    from concourse import bacc

    dt_mm = getattr(mybir.dt, MM_DTYPE)
    f32 = mybir.dt.float32
    D_SHARD = D_IN // N_CORES       # 4096 contraction rows per core
    DC = D_SHARD // P               # 32 d-chunks
    NB = N_TOK // 512               # 16 moving n-blocks
    OC = D_OUT // P                 # 6 output-channel chunks

    nc = bacc.Bacc(None, target_bir_lowering=False)
    xT = nc.dram_tensor("xT", [D_SHARD, N_TOK], dt_mm, kind="ExternalInput")
    wT = nc.dram_tensor("wT", [D_SHARD, D_OUT], dt_mm, kind="ExternalInput")
    outT = nc.dram_tensor("outT", [D_OUT, N_TOK], f32, kind="ExternalOutput")

    with tile.TileContext(nc) as tc:
        with (
            tc.tile_pool(name="w", bufs=1) as wpool,
            tc.tile_pool(name="xt", bufs=4) as xpool,
            tc.tile_pool(name="ot", bufs=4) as opool,
            tc.tile_pool(name="psum", bufs=1, space="PSUM") as ppool,
        ):
            ws = wpool.tile([P, DC, D_OUT], dt_mm, name="ws")
            for j in range(DC):
                nc.sync.dma_start(ws[:, j, :], wT[j * P:(j + 1) * P, :])
            for nb in range(NB):
                psums = [
                    ppool.tile([P, 512], f32, name=f"psum{i}")
                    for i in range(OC)
                ]
                for dc in range(DC):
                    xt = xpool.tile([P, 512], dt_mm, name="xt")
                    nc.sync.dma_start(
                        xt[:], xT[dc * P:(dc + 1) * P, nb * 512:(nb + 1) * 512]
                    )
                    st = dc == 0
                    sp = dc == DC - 1
                    for oc in range(OC):
                        nc.tensor.matmul(
                            psums[oc][:], ws[:, dc, oc * P:(oc + 1) * P],
                            xt[:], start=st, stop=sp,
                        )
                for oc in range(OC):
                    ot = opool.tile([P, 512], f32, name="ot")
                    nc.vector.tensor_copy(ot[:], psums[oc][:])
                    nc.sync.dma_start(
                        outT[oc * P:(oc + 1) * P, nb * 512:(nb + 1) * 512],
                        ot[:],
                    )

    nc.compile()
    return nc


def _build_bass():
    if DESIGN == "sbuf":
        return _build_bass_sbuf()
    if DESIGN == "dp_ot":
        return _build_bass_dp_ot()
    if DESIGN == "kshard":
        return _build_bass_kshard()
    if DESIGN == "kshard_ot":
        return _build_bass_kshard_ot()
    import concourse.mybir as mybir
    import concourse.tile as tile
    from concourse import bacc

    dt_mm = getattr(mybir.dt, MM_DTYPE)
    f32 = mybir.dt.float32

    nc = bacc.Bacc(None, target_bir_lowering=False)
    xT = nc.dram_tensor("xT", [D_IN, N_SHARD], dt_mm, kind="ExternalInput")
    wT = nc.dram_tensor("wT", [D_IN, D_OUT], dt_mm, kind="ExternalInput")
    out = nc.dram_tensor("out", [N_SHARD, D_OUT], f32, kind="ExternalOutput")

    with tile.TileContext(nc) as tc:
        with (
            tc.tile_pool(name="xt", bufs=4) as xpool,
            tc.tile_pool(name="wt", bufs=4) as wpool,
            tc.tile_pool(name="ot", bufs=4) as opool,
            tc.tile_pool(name="psum", bufs=1, space="PSUM") as ppool,
        ):
            for ns in range(N_SUPERS):
                psums = [
                    ppool.tile([P, D_OUT], f32, name=f"psum{i}")
                    for i in range(N_CH)
                ]
                for dc in range(D_CHUNKS):
                    xt = xpool.tile([P, N_SUPER], dt_mm)
                    wt = wpool.tile([P, D_OUT], dt_mm)
                    nc.sync.dma_start(
                        xt[:],
                        xT[dc * P:(dc + 1) * P, ns * N_SUPER:(ns + 1) * N_SUPER],
                    )
                    nc.sync.dma_start(wt[:], wT[dc * P:(dc + 1) * P, :])
                    st = dc == 0
                    sp = dc == D_CHUNKS - 1
                    for nch in range(N_CH):
                        lhsT = xt[:, nch * P:(nch + 1) * P]
                        nc.tensor.matmul(
                            psums[nch][:, 0:512], lhsT, wt[:, 0:512],
                            start=st, stop=sp,
                        )
                        nc.tensor.matmul(
                            psums[nch][:, 512:D_OUT], lhsT, wt[:, 512:D_OUT],
                            start=st, stop=sp,
                        )
                for nch in range(N_CH):
                    ot = opool.tile([P, D_OUT], f32)
                    nc.vector.tensor_copy(ot[:], psums[nch][:])
                    base = ns * N_SUPER + nch * P
                    nc.sync.dma_start(out[base:base + P, :], ot[:])

    nc.compile()
    return nc


def kernel(x: np.ndarray, W: np.ndarray, b_pre: np.ndarray) -> np.ndarray:
    global MM_DTYPE

    x = np.asarray(x, dtype=np.float32)
    W = np.asarray(W, dtype=np.float32)
    b_pre = np.asarray(b_pre, dtype=np.float32)

    # Fold the pre-bias on the host (exact no-op for b_pre == 0).
    if b_pre.any():
        x = x - b_pre[None, :]

    out = _run_device(x, W)

    # Cheap sampled sanity check (64 rows vs numpy fp64). Expected
    # scale-relative error: ~1.2e-2 for bf16+fp8 hybrid (F8=12),
    # ~1.5e-3 for pure bfloat16, ~1.6e-4 for float32r. Anything above
    # the gate means the fast path misbehaved on this machine -> retry
    # one tier more exact (drop fp8 first, then drop bf16).
    idx = np.arange(0, N_TOK, N_TOK // 64)
    ref = x[idx].astype(np.float64) @ W.astype(np.float64).T
    err = np.abs(out[idx] - ref).max() / (np.abs(ref).max() + 1e-30)
    gate = 1.55e-2 if _f8_pairs() else _ERR_GATE.get(MM_DTYPE, 5e-3)
    if not np.isfinite(err) or err > gate:
        if _f8_pairs():
            _F8_STATE[0] = 0
            out = kernel(x, W, np.zeros_like(b_pre))
        elif MM_DTYPE != "float32":
            MM_DTYPE = "float32r" if MM_DTYPE in ("bfloat16", "float16") \
                else "float32"
            out = kernel(x, W, np.zeros_like(b_pre))
    return out


def _run_device(x: np.ndarray, W: np.ndarray) -> np.ndarray:
    global LAST_RESULTS
    from concourse.bass_utils import run_bass_kernel_spmd

    x8 = W8 = None
    F8 = _f8_pairs() if DESIGN == "sbuf" else 0
    if MM_DTYPE in ("bfloat16", "float16"):
        import ml_dtypes

        host_dt = np.dtype(getattr(ml_dtypes, MM_DTYPE))
        if F8:
            DB = D_IN - 2 * F8 * P
            f8dt = np.dtype(ml_dtypes.float8_e4m3)
            x8 = x[:, DB:].astype(f8dt)
            W8 = (W[:, DB:] * W8_SCALE).astype(f8dt)
            x = x[:, :DB].astype(host_dt)
            W = W[:, :DB].astype(host_dt)
        else:
            x = x.astype(host_dt)
            W = W.astype(host_dt)

    wTc = np.ascontiguousarray(W.T)  # [D_IN(-D8), D_OUT]
    if DESIGN in ("kshard", "kshard_ot"):
        D_SHARD = D_IN // N_CORES
        xTfull = np.ascontiguousarray(x.T)  # [D_IN, N_TOK]
        in_maps = [{
            "xT": xTfull[c * D_SHARD:(c + 1) * D_SHARD],
            "wT": wTc[c * D_SHARD:(c + 1) * D_SHARD],
        } for c in range(N_CORES)]
    else:
        in_maps = [{
            "xT": np.ascontiguousarray(x[c * N_SHARD:(c + 1) * N_SHARD].T),
            "wT": wTc,
        } for c in range(N_CORES)]
        if F8:
            w8Tc = np.ascontiguousarray(W8.T)
            for c in range(N_CORES):
                in_maps[c]["x8T"] = np.ascontiguousarray(
                    x8[c * N_SHARD:(c + 1) * N_SHARD].T
                )
                in_maps[c]["w8T"] = w8Tc

    nc = _build_bass()
    last_err = None
    for attempt in range(3):
        try:
            LAST_RESULTS = run_bass_kernel_spmd(
                nc, in_maps, core_ids=list(range(N_CORES)),
                tmpdir=os.environ.get("KERNEL_TRACE_DIR") or None,
            )
            break
        except Exception as e:  # transient device faults recover on retry
            last_err = e
            import time

            time.sleep(10)
    else:
        raise last_err
    if DESIGN == "kshard":
        # Tensor-parallel: reduce the per-core partials (host all-reduce).
        acc = np.zeros((N_TOK, D_OUT), dtype=np.float64)
        for c in range(N_CORES):
            acc += LAST_RESULTS.results[c]["out"]
        out = acc.astype(np.float32)
    elif DESIGN == "kshard_ot":
        acc = np.zeros((D_OUT, N_TOK), dtype=np.float64)
        for c in range(N_CORES):
            acc += LAST_RESULTS.results[c]["outT"]
        out = np.ascontiguousarray(acc.T.astype(np.float32))
    elif DESIGN == "dp_ot":
        out = np.concatenate(
            [
                np.ascontiguousarray(LAST_RESULTS.results[c]["outT"].T)
                for c in range(N_CORES)
            ],
            axis=0,
        )
    else:
        out = np.concatenate(
            [LAST_RESULTS.results[c]["out"] for c in range(N_CORES)], axis=0
        )
    return out



# revision 16
# speedup vs baseline: 1.1927x; 1.1927x over previous
"""Trainium2 Bass kernel for nn_Decoder: out = (x - b_pre) @ W^T.

Shapes (hardcoded): x [8192, 32768] f32, W [768, 32768] f32, b_pre [32768] f32
-> out [8192, 768] f32.

Sharding: data-parallel over the 8192 token rows across 8 NeuronCores
(1024 rows each), W replicated. The TensorE contracts over the partition
axis, so both operands are fed with the contraction dim (d = 32768) on
partitions: the host pre-transposes each x shard to xT [32768, 1024] and
W to wT [32768, 768] (cheap, ~2 s total). b_pre is folded into x on the
host (x - b_pre) before the transpose; with the reference's b_pre == 0
this is bitwise a no-op.

Default per-core kernel (DESIGN="sbuf", MM_DTYPE="float32r"): stream d
in 32 supers of 8x128 rows; each super DMAs 8 xT chunks [128, 1024] and
8 wT chunks [128, 768] (both tensors touch HBM exactly once, 227 MiB).
For each of 8 output row-chunks, 16 matmuls accumulate the super's
contraction into a [128, 768] PSUM tile (x chunk as the self-loading
stationary operand, wT as the 512/256-col moving operand), and the DVE
adds the PSUM tile into the SBUF-resident [1024, 768] output. x DMAs
issue from the SP HWDGE engine and W DMAs from ACT, halving per-engine
descriptor-issue load. Measured: 758 us HW at 95.7% PE-matmul
occupancy; float32r streams moving columns at ~9/8 cyc/col, so the PE
floor is 737 us and the structural floor (plus ~24 us fixed Tile
preamble/drain) is ~760 us. Scale-relative error 1.6e-4 vs fp64.
"float32" mode is exact (1e-6) at ~2.66 ms.

Tuning notes: DSUP=16 regresses (prefetch margin too thin -> PE input
waits + cold clock); XT/WT_BUFS=22 overflows SBUF; single-engine DMA
issue costs ~9 us; 16/16/3 + dual-engine issue is the optimum found.
"""

import os
import sys

if "/opt/trn_rl_repo" not in sys.path:
    sys.path.insert(0, "/opt/trn_rl_repo")

import numpy as np


def _shim_axon_hooks():
    """The agent image's `antenv` package lacks `axon_hooks`; when a
    caller sets BASS_TRACE=1 the bass trace path would ImportError. Shim
    the module (registering the same ctypes NTFF hook trn_boot would
    have) so tracing degrades gracefully instead of crashing."""
    try:
        import antenv.axon_hooks  # noqa: F401
        return
    except ImportError:
        pass
    try:
        import types

        import antenv
        from trn_agent_boot.trn_boot import _ntff_profile_via_ctypes

        m = types.ModuleType("antenv.axon_hooks")
        m._hook = _ntff_profile_via_ctypes("/opt/axon/libaxon_pjrt.so")
        m.get_axon_ntff_profile_hook = lambda: m._hook
        m.set_axon_ntff_profile_hook = lambda h: setattr(m, "_hook", h)
        sys.modules["antenv.axon_hooks"] = m
        antenv.axon_hooks = m
    except Exception:
        pass


_shim_axon_hooks()

N_TOK = 8192
D_IN = 32768
D_OUT = 768
N_CORES = 8
N_SHARD = N_TOK // N_CORES          # 1024 token rows per core
P = 128
D_CHUNKS = D_IN // P                # 256
N_SUPER = 512                       # token rows resident in PSUM at once
N_SUPERS = N_SHARD // N_SUPER       # 2
N_CH = N_SUPER // P                 # 4 psum tiles per n-block

# Matmul input dtype knob:
#   "bfloat16": 1 cyc/col on the PE (78.6 TF/s/core) and half the HBM
#       traffic of fp32. Scale-relative error ~1.5e-3 at K=32768 --
#       well under the 2e-2 gate.
#   "float32r": single-pass fp32 PE matmul, ~9/8 cyc/col measured, max
#       scale-relative error 1.6e-4.
#   "float32": exact to 1e-6 but 4 cyc/col -> ~3.5x slower.
MM_DTYPE = os.environ.get("KERNEL_MM_DTYPE", "bfloat16")

# Max acceptable scale-relative error (vs fp64, 64 sampled rows) before
# kernel() falls back to a more exact dtype.
_ERR_GATE = {"bfloat16": 8e-3, "float16": 8e-3, "float32r": 5e-3,
             "float32": 1e-4}
# "sbuf": d-super blocking, output accumulated in SBUF, min DMA traffic
#         (best: 766 us HW).
# "psum": full-K accumulation in PSUM, W streamed twice (simplest).
# "kshard"/"kshard_ot": tensor-parallel over the contraction dim.
DESIGN = os.environ.get("KERNEL_DESIGN", "sbuf")

LAST_RESULTS = None  # BassKernelResults of the most recent kernel() call


W8_SCALE = 128.0  # host pre-scales the fp8 W shard into e4m3 normal range


def _f8_pairs() -> int:
    """Chunk-PAIRS (2x128 K rows each) computed in fp8 DoubleRow. Only
    meaningful for the bfloat16 fast path."""
    if MM_DTYPE != "bfloat16":
        return 0
    return _F8_STATE[0]


_F8_STATE = [int(os.environ.get("KERNEL_F8_PAIRS", "16"))]


def _build_bass_sbuf():
    """Design 1: stream xT and wT exactly once in d-supers; accumulate
    the [1024, 768] output in SBUF across d-supers (DVE adds PSUM into
    the resident C tiles).

    Refinements over the fp32r baseline:
    - warmup ramp: the first two supers are half-size (4 chunks) so the
      first matmuls don't outrun the just-started DMA stream;
    - optional fp8 tail: the last 2*F8 d-chunks are computed in
      DoubleRow fp8e4 (2x PE rate, K=256 per matmul). W is pre-scaled
      by W8_SCALE on the host; the drain folds 1/W8_SCALE back in via
      scalar_tensor_tensor (ct = ps*(1/s) + ct);
    - per-chunk output DMAs issued right after each chunk's last drain,
      alternating the sync/scalar issue queues, so the writeback
      overlaps the remaining compute."""
    import concourse.mybir as mybir
    import concourse.tile as tile
    from concourse import bacc

    dt_mm = getattr(mybir.dt, MM_DTYPE)
    f32 = mybir.dt.float32
    f8 = mybir.dt.float8e4
    F8 = _f8_pairs()
    D8 = 2 * F8 * P                # fp8 K rows (tail of the d axis)
    DB = D_IN - D8                 # bf16/fp32 K rows
    CHB = DB // P                  # full-precision d-chunks
    DSUP = int(os.environ.get("KERNEL_DSUP", "8"))  # d-chunks per super
    sched = [DSUP] * (CHB // DSUP)
    assert sum(sched) == CHB, (sched, CHB)
    NCH = N_SHARD // P             # 8 output row-chunks

    nc = bacc.Bacc(None, target_bir_lowering=False)
    xT = nc.dram_tensor("xT", [DB, N_SHARD], dt_mm, kind="ExternalInput")
    wT = nc.dram_tensor("wT", [DB, D_OUT], dt_mm, kind="ExternalInput")
    if F8:
        x8T = nc.dram_tensor("x8T", [D8, N_SHARD], f8, kind="ExternalInput")
        w8T = nc.dram_tensor("w8T", [D8, D_OUT], f8, kind="ExternalInput")
    out = nc.dram_tensor("out", [N_SHARD, D_OUT], f32, kind="ExternalOutput")

    XT_BUFS = int(os.environ.get("KERNEL_XT_BUFS", "24"))
    WT_BUFS = int(os.environ.get("KERNEL_WT_BUFS", "24"))
    # DoubleRow-mode psum tiles are charged 2x banks by the allocator,
    # so with the fp8 tail only 2 bufs fit (2x4 banks); still enough to
    # hide each drain under the next matmul group.
    PS_BUFS = int(os.environ.get("KERNEL_PS_BUFS", "2" if F8 else "3"))
    mul = mybir.AluOpType.mult
    add = mybir.AluOpType.add
    with tile.TileContext(nc) as tc:
        with (
            tc.tile_pool(name="xs", bufs=XT_BUFS) as xpool,
            tc.tile_pool(name="ws", bufs=WT_BUFS) as wpool,
            tc.tile_pool(name="c", bufs=1) as cpool,
            tc.tile_pool(name="f8x", bufs=max(F8, 1)) as f8xpool,
            tc.tile_pool(name="f8w", bufs=max(F8, 1)) as f8wpool,
            tc.tile_pool(name="psum", bufs=PS_BUFS, space="PSUM") as ppool,
        ):
            cts = [
                cpool.tile([P, D_OUT], f32, name=f"c{i}") for i in range(NCH)
            ]
            x8ts = []
            w8ts = []
            chunk0 = 0
            for si, DS in enumerate(sched):
                # Per-chunk tiles (not one slab) so the first matmul of a
                # super only waits on one chunk DMA, and prefetch runs
                # chunk-granular across supers.
                xts = []
                wts = []
                for j in range(DS):
                    row = (chunk0 + j) * P
                    xt = xpool.tile([P, N_SHARD], dt_mm, name="xt")
                    wt = wpool.tile([P, D_OUT], dt_mm, name="wt")
                    # Split descriptor issue across the two HWDGE engines
                    # (SP + ACT) so x and W prefetch don't queue behind
                    # each other on one issue path.
                    nc.sync.dma_start(xt[:], xT[row:row + P, :])
                    nc.scalar.dma_start(wt[:], wT[row:row + P, :])
                    xts.append(xt)
                    wts.append(wt)
                if si == len(sched) - 1 and F8:
                    # fp8 tail operands: issued after all bf16 prefetch
                    # DMAs so they never starve the main stream; they
                    # land well before the fp8 section runs.
                    for p8 in range(F8):
                        x8t = f8xpool.tile([P, 2, N_SHARD], f8, name="x8t")
                        w8t = f8wpool.tile([P, 2, D_OUT], f8, name="w8t")
                        for ks in range(2):
                            row = (2 * p8 + ks) * P
                            nc.sync.dma_start(x8t[:, ks, :],
                                              x8T[row:row + P, :])
                            nc.scalar.dma_start(w8t[:, ks, :],
                                                w8T[row:row + P, :])
                        x8ts.append(x8t)
                        w8ts.append(w8t)
                for nch in range(NCH):
                    ps = ppool.tile([P, D_OUT], f32, name="ps")
                    for j in range(DS):
                        lhsT = xts[j][:, nch * P:(nch + 1) * P]
                        nc.tensor.matmul(
                            ps[:, 0:512], lhsT, wts[j][:, 0:512],
                            start=(j == 0), stop=(j == DS - 1),
                        )
                        nc.tensor.matmul(
                            ps[:, 512:D_OUT], lhsT, wts[j][:, 512:D_OUT],
                            start=(j == 0), stop=(j == DS - 1),
                        )
                    if si == 0:
                        nc.vector.tensor_copy(cts[nch][:], ps[:])
                    else:
                        nc.vector.tensor_add(cts[nch][:], cts[nch][:], ps[:])
                    if si == len(sched) - 1 and not F8:
                        eng = nc.sync if nch % 2 == 0 else nc.scalar
                        eng.dma_start(out[nch * P:(nch + 1) * P, :],
                                      cts[nch][:])
                chunk0 += DS
            if F8:
                dr = mybir.MatmulPerfMode.DoubleRow
                for nch in range(NCH):
                    ps = ppool.tile([P, D_OUT], f32, name="ps8")
                    for p8 in range(F8):
                        lhsT = x8ts[p8][:, :, nch * P:(nch + 1) * P]
                        nc.tensor.matmul(
                            ps[:, 0:512], lhsT, w8ts[p8][:, :, 0:512],
                            start=(p8 == 0), stop=(p8 == F8 - 1),
                            perf_mode=dr,
                        )
                        nc.tensor.matmul(
                            ps[:, 512:D_OUT], lhsT,
                            w8ts[p8][:, :, 512:D_OUT],
                            start=(p8 == 0), stop=(p8 == F8 - 1),
                            perf_mode=dr,
                        )
                    nc.vector.scalar_tensor_tensor(
                        cts[nch][:], ps[:], 1.0 / W8_SCALE, cts[nch][:],
                        op0=mul, op1=add,
                    )
                    eng = nc.sync if nch % 2 == 0 else nc.scalar
                    eng.dma_start(out[nch * P:(nch + 1) * P, :], cts[nch][:])

    nc.compile()
    return nc


def _build_bass_dp_ot():
    """Design 5 (data-parallel, W-stationary, all-512 moving): like
    "sbuf" but with wT chunks as the stationary operand and xT as the
    512-wide moving operand, so every matmul streams 512 cols (213 ns at
    bf16) and fully hides its ~97 ns LDWEIGHTS (the x-stationary 256-col
    matmuls at 107 ns barely cover it). Output accumulates in SBUF
    transposed [768, N_SHARD]; the host transposes back at gather. Final
    out DMAs alternate sync/scalar queues and are issued per-chunk so the
    drain overlaps the last super's adds."""
    import concourse.mybir as mybir
    import concourse.tile as tile
    from concourse import bacc

    dt_mm = getattr(mybir.dt, MM_DTYPE)
    f32 = mybir.dt.float32
    DSUP = int(os.environ.get("KERNEL_DSUP", "8"))  # d-chunks per super
    NSUP = D_CHUNKS // DSUP        # supers
    OC = D_OUT // P                # 6 output-channel chunks
    TH = N_SHARD // 512            # 2 moving halves of 512 tokens

    nc = bacc.Bacc(None, target_bir_lowering=False)
    xT = nc.dram_tensor("xT", [D_IN, N_SHARD], dt_mm, kind="ExternalInput")
    wT = nc.dram_tensor("wT", [D_IN, D_OUT], dt_mm, kind="ExternalInput")
    outT = nc.dram_tensor("outT", [D_OUT, N_SHARD], f32, kind="ExternalOutput")

    XT_BUFS = int(os.environ.get("KERNEL_XT_BUFS", "16"))
    WT_BUFS = int(os.environ.get("KERNEL_WT_BUFS", "16"))
    PS_BUFS = int(os.environ.get("KERNEL_PS_BUFS", "3"))
    with tile.TileContext(nc) as tc:
        with (
            tc.tile_pool(name="xs", bufs=XT_BUFS) as xpool,
            tc.tile_pool(name="ws", bufs=WT_BUFS) as wpool,
            tc.tile_pool(name="c", bufs=1) as cpool,
            tc.tile_pool(name="psum", bufs=PS_BUFS, space="PSUM") as ppool,
        ):
            cts = [
                cpool.tile([P, N_SHARD], f32, name=f"c{i}") for i in range(OC)
            ]
            for ds in range(NSUP):
                xts = []
                wts = []
                for j in range(DSUP):
                    row = (ds * DSUP + j) * P
                    xt = xpool.tile([P, N_SHARD], dt_mm, name="xt")
                    wt = wpool.tile([P, D_OUT], dt_mm, name="wt")
                    nc.sync.dma_start(xt[:], xT[row:row + P, :])
                    nc.scalar.dma_start(wt[:], wT[row:row + P, :])
                    xts.append(xt)
                    wts.append(wt)
                for oc in range(OC):
                    for th in range(TH):
                        ps = ppool.tile([P, 512], f32, name="ps")
                        for j in range(DSUP):
                            nc.tensor.matmul(
                                ps[:],
                                wts[j][:, oc * P:(oc + 1) * P],
                                xts[j][:, th * 512:(th + 1) * 512],
                                start=(j == 0), stop=(j == DSUP - 1),
                            )
                        dst = cts[oc][:, th * 512:(th + 1) * 512]
                        if ds == 0:
                            nc.vector.tensor_copy(dst, ps[:])
                        else:
                            nc.vector.tensor_add(dst, dst, ps[:])
                    if ds == NSUP - 1:
                        eng = nc.sync if oc % 2 == 0 else nc.scalar
                        eng.dma_start(outT[oc * P:(oc + 1) * P, :], cts[oc][:])

    nc.compile()
    return nc


def _build_bass_kshard():
    """Design 3 (tensor-parallel): shard the contraction dim d across
    cores (4096 rows each). The W^T shard [4096, 768] (12 MiB) stays
    resident in SBUF; x^T [4096, 8192] streams through once. Each core
    produces a full [8192, 768] partial; the host reduces the 8 partials
    at gather time (the sharding hint's "all-reduce on the [N,768]
    output"). PSUM accumulates the core's entire local contraction."""
    import concourse.mybir as mybir
    import concourse.tile as tile
    from concourse import bacc

    dt_mm = getattr(mybir.dt, MM_DTYPE)
    f32 = mybir.dt.float32
    D_SHARD = D_IN // N_CORES       # 4096 contraction rows per core
    DC = D_SHARD // P               # 32 d-chunks
    NB = N_TOK // N_SUPER           # 16 n-blocks of 512 token rows

    nc = bacc.Bacc(None, target_bir_lowering=False)
    xT = nc.dram_tensor("xT", [D_SHARD, N_TOK], dt_mm, kind="ExternalInput")
    wT = nc.dram_tensor("wT", [D_SHARD, D_OUT], dt_mm, kind="ExternalInput")
    out = nc.dram_tensor("out", [N_TOK, D_OUT], f32, kind="ExternalOutput")

    with tile.TileContext(nc) as tc:
        with (
            tc.tile_pool(name="w", bufs=1) as wpool,
            tc.tile_pool(name="xt", bufs=4) as xpool,
            tc.tile_pool(name="ot", bufs=4) as opool,
            tc.tile_pool(name="psum", bufs=1, space="PSUM") as ppool,
        ):
            ws = wpool.tile([P, DC, D_OUT], dt_mm, name="ws")
            for j in range(DC):
                nc.sync.dma_start(ws[:, j, :], wT[j * P:(j + 1) * P, :])
            for nb in range(NB):
                psums = [
                    ppool.tile([P, D_OUT], f32, name=f"psum{i}")
                    for i in range(N_CH)
                ]
                for dc in range(DC):
                    xt = xpool.tile([P, N_SUPER], dt_mm, name="xt")
                    nc.sync.dma_start(
                        xt[:],
                        xT[dc * P:(dc + 1) * P,
                           nb * N_SUPER:(nb + 1) * N_SUPER],
                    )
                    st = dc == 0
                    sp = dc == DC - 1
                    for nch in range(N_CH):
                        lhsT = xt[:, nch * P:(nch + 1) * P]
                        nc.tensor.matmul(
                            psums[nch][:, 0:512], lhsT, ws[:, dc, 0:512],
                            start=st, stop=sp,
                        )
                        nc.tensor.matmul(
                            psums[nch][:, 512:D_OUT], lhsT,
                            ws[:, dc, 512:D_OUT],
                            start=st, stop=sp,
                        )
                for nch in range(N_CH):
                    ot = opool.tile([P, D_OUT], f32, name="ot")
                    nc.vector.tensor_copy(ot[:], psums[nch][:])
                    base = nb * N_SUPER + nch * P
                    nc.sync.dma_start(out[base:base + P, :], ot[:])

    nc.compile()
    return nc


def _build_bass_kshard_ot():
    """Design 4 (tensor-parallel, W-stationary): like kshard, but W^T
    tiles are the stationary operand and x^T streams as the moving side,
    so every matmul has a 512-wide moving operand. For float32r each
    matmul self-loads its stationary via a ~214 ns LDWEIGHTS; with all
    matmuls at N=512 (213 ns) the loads pipeline behind the previous
    matmul instead of stalling (the N=256 matmuls of the x-stationary
    designs are LDW-bound). Output lands transposed [768, 8192]; the
    host transposes back during the reduce."""
    import concourse.mybir as mybir
    import concourse.tile as tile
    from concourse import bacc

    dt_mm = getattr(mybir.dt, MM_DTYPE)
    f32 = mybir.dt.float32
    D_SHARD = D_IN // N_CORES       # 4096 contraction rows per core
    DC = D_SHARD // P               # 32 d-chunks
    NB = N_TOK // 512               # 16 moving n-blocks
    OC = D_OUT // P                 # 6 output-channel chunks

    nc = bacc.Bacc(None, target_bir_lowering=False)
    xT = nc.dram_tensor("xT", [D_SHARD, N_TOK], dt_mm, kind="ExternalInput")
    wT = nc.dram_tensor("wT", [D_SHARD, D_OUT], dt_mm, kind="ExternalInput")
    outT = nc.dram_tensor("outT", [D_OUT, N_TOK], f32, kind="ExternalOutput")

    with tile.TileContext(nc) as tc:
        with (
            tc.tile_pool(name="w", bufs=1) as wpool,
            tc.tile_pool(name="xt", bufs=4) as xpool,
            tc.tile_pool(name="ot", bufs=4) as opool,
            tc.tile_pool(name="psum", bufs=1, space="PSUM") as ppool,
        ):
            ws = wpool.tile([P, DC, D_OUT], dt_mm, name="ws")
            for j in range(DC):
                nc.sync.dma_start(ws[:, j, :], wT[j * P:(j + 1) * P, :])
            for nb in range(NB):
                psums = [
                    ppool.tile([P, 512], f32, name=f"psum{i}")
                    for i in range(OC)
                ]
                for dc in range(DC):
                    xt = xpool.tile([P, 512], dt_mm, name="xt")
                    nc.sync.dma_start(
                        xt[:], xT[dc * P:(dc + 1) * P, nb * 512:(nb + 1) * 512]
                    )
                    st = dc == 0
                    sp = dc == DC - 1
                    for oc in range(OC):
                        nc.tensor.matmul(
                            psums[oc][:], ws[:, dc, oc * P:(oc + 1) * P],
                            xt[:], start=st, stop=sp,
                        )
                for oc in range(OC):
                    ot = opool.tile([P, 512], f32, name="ot")
                    nc.vector.tensor_copy(ot[:], psums[oc][:])
                    nc.sync.dma_start(
                        outT[oc * P:(oc + 1) * P, nb * 512:(nb + 1) * 512],
                        ot[:],
                    )

    nc.compile()
    return nc


def _build_bass():
    if DESIGN == "sbuf":
        return _build_bass_sbuf()
    if DESIGN == "dp_ot":
        return _build_bass_dp_ot()
    if DESIGN == "kshard":
        return _build_bass_kshard()
    if DESIGN == "kshard_ot":
        return _build_bass_kshard_ot()
    import concourse.mybir as mybir
    import concourse.tile as tile
    from concourse import bacc

    dt_mm = getattr(mybir.dt, MM_DTYPE)
    f32 = mybir.dt.float32

    nc = bacc.Bacc(None, target_bir_lowering=False)
    xT = nc.dram_tensor("xT", [D_IN, N_SHARD], dt_mm, kind="ExternalInput")
    wT = nc.dram_tensor("wT", [D_IN, D_OUT], dt_mm, kind="ExternalInput")
    out = nc.dram_tensor("out", [N_SHARD, D_OUT], f32, kind="ExternalOutput")

    with tile.TileContext(nc) as tc:
        with (
            tc.tile_pool(name="xt", bufs=4) as xpool,
            tc.tile_pool(name="wt", bufs=4) as wpool,
            tc.tile_pool(name="ot", bufs=4) as opool,
            tc.tile_pool(name="psum", bufs=1, space="PSUM") as ppool,
        ):
            for ns in range(N_SUPERS):
                psums = [
                    ppool.tile([P, D_OUT], f32, name=f"psum{i}")
                    for i in range(N_CH)
                ]
                for dc in range(D_CHUNKS):
                    xt = xpool.tile([P, N_SUPER], dt_mm)
                    wt = wpool.tile([P, D_OUT], dt_mm)
                    nc.sync.dma_start(
                        xt[:],
                        xT[dc * P:(dc + 1) * P, ns * N_SUPER:(ns + 1) * N_SUPER],
                    )
                    nc.sync.dma_start(wt[:], wT[dc * P:(dc + 1) * P, :])
                    st = dc == 0
                    sp = dc == D_CHUNKS - 1
                    for nch in range(N_CH):
                        lhsT = xt[:, nch * P:(nch + 1) * P]
                        nc.tensor.matmul(
                            psums[nch][:, 0:512], lhsT, wt[:, 0:512],
                            start=st, stop=sp,
                        )
                        nc.tensor.matmul(
                            psums[nch][:, 512:D_OUT], lhsT, wt[:, 512:D_OUT],
                            start=st, stop=sp,
                        )
                for nch in range(N_CH):
                    ot = opool.tile([P, D_OUT], f32)
                    nc.vector.tensor_copy(ot[:], psums[nch][:])
                    base = ns * N_SUPER + nch * P
                    nc.sync.dma_start(out[base:base + P, :], ot[:])

    nc.compile()
    return nc


def kernel(x: np.ndarray, W: np.ndarray, b_pre: np.ndarray) -> np.ndarray:
    global MM_DTYPE

    x = np.asarray(x, dtype=np.float32)
    W = np.asarray(W, dtype=np.float32)
    b_pre = np.asarray(b_pre, dtype=np.float32)

    # Fold the pre-bias on the host (exact no-op for b_pre == 0).
    if b_pre.any():
        x = x - b_pre[None, :]

    out = _run_device(x, W)

    # Cheap sampled sanity check (64 rows vs numpy fp64). Expected
    # scale-relative error: ~1.2e-2 for bf16+fp8 hybrid (F8=12),
    # ~1.5e-3 for pure bfloat16, ~1.6e-4 for float32r. Anything above
    # the gate means the fast path misbehaved on this machine -> retry
    # one tier more exact (drop fp8 first, then drop bf16).
    idx = np.arange(0, N_TOK, N_TOK // 64)
    ref = x[idx].astype(np.float64) @ W.astype(np.float64).T
    err = np.abs(out[idx] - ref).max() / (np.abs(ref).max() + 1e-30)
    gate = 1.7e-2 if _f8_pairs() else _ERR_GATE.get(MM_DTYPE, 5e-3)
    if not np.isfinite(err) or err > gate:
        if _f8_pairs():
            _F8_STATE[0] = 0
            out = kernel(x, W, np.zeros_like(b_pre))
        elif MM_DTYPE != "float32":
            MM_DTYPE = "float32r" if MM_DTYPE in ("bfloat16", "float16") \
                else "float32"
            out = kernel(x, W, np.zeros_like(b_pre))
    return out


def _run_device(x: np.ndarray, W: np.ndarray) -> np.ndarray:
    global LAST_RESULTS
    from concourse.bass_utils import run_bass_kernel_spmd

    x8 = W8 = None
    F8 = _f8_pairs() if DESIGN == "sbuf" else 0
    if MM_DTYPE in ("bfloat16", "float16"):
        import ml_dtypes

        host_dt = np.dtype(getattr(ml_dtypes, MM_DTYPE))
        if F8:
            DB = D_IN - 2 * F8 * P
            f8dt = np.dtype(ml_dtypes.float8_e4m3)
            x8 = x[:, DB:].astype(f8dt)
            W8 = (W[:, DB:] * W8_SCALE).astype(f8dt)
            x = x[:, :DB].astype(host_dt)
            W = W[:, :DB].astype(host_dt)
        else:
            x = x.astype(host_dt)
            W = W.astype(host_dt)

    wTc = np.ascontiguousarray(W.T)  # [D_IN(-D8), D_OUT]
    if DESIGN in ("kshard", "kshard_ot"):
        D_SHARD = D_IN // N_CORES
        xTfull = np.ascontiguousarray(x.T)  # [D_IN, N_TOK]
        in_maps = [{
            "xT": xTfull[c * D_SHARD:(c + 1) * D_SHARD],
            "wT": wTc[c * D_SHARD:(c + 1) * D_SHARD],
        } for c in range(N_CORES)]
    else:
        in_maps = [{
            "xT": np.ascontiguousarray(x[c * N_SHARD:(c + 1) * N_SHARD].T),
            "wT": wTc,
        } for c in range(N_CORES)]
        if F8:
            w8Tc = np.ascontiguousarray(W8.T)
            for c in range(N_CORES):
                in_maps[c]["x8T"] = np.ascontiguousarray(
                    x8[c * N_SHARD:(c + 1) * N_SHARD].T
                )
                in_maps[c]["w8T"] = w8Tc

    nc = _build_bass()
    last_err = None
    for attempt in range(3):
        try:
            LAST_RESULTS = run_bass_kernel_spmd(
                nc, in_maps, core_ids=list(range(N_CORES)),
                tmpdir=os.environ.get("KERNEL_TRACE_DIR") or None,
            )
            break
        except Exception as e:  # transient device faults recover on retry
            last_err = e
            import time

            time.sleep(10)
    else:
        raise last_err
    if DESIGN == "kshard":
        # Tensor-parallel: reduce the per-core partials (host all-reduce).
        acc = np.zeros((N_TOK, D_OUT), dtype=np.float64)
        for c in range(N_CORES):
            acc += LAST_RESULTS.results[c]["out"]
        out = acc.astype(np.float32)
    elif DESIGN == "kshard_ot":
        acc = np.zeros((D_OUT, N_TOK), dtype=np.float64)
        for c in range(N_CORES):
            acc += LAST_RESULTS.results[c]["outT"]
        out = np.ascontiguousarray(acc.T.astype(np.float32))
    elif DESIGN == "dp_ot":
        out = np.concatenate(
            [
                np.ascontiguousarray(LAST_RESULTS.results[c]["outT"].T)
                for c in range(N_CORES)
            ],
            axis=0,
        )
    else:
        out = np.concatenate(
            [LAST_RESULTS.results[c]["out"] for c in range(N_CORES)], axis=0
        )
    return out



# revision 20
# speedup vs baseline: 1.2003x; 1.0064x over previous
"""Trainium2 Bass kernel for nn_Decoder: out = (x - b_pre) @ W^T.

Shapes (hardcoded): x [8192, 32768] f32, W [768, 32768] f32, b_pre [32768] f32
-> out [8192, 768] f32.

Sharding: data-parallel over the 8192 token rows across 8 NeuronCores
(1024 rows each), W replicated. The TensorE contracts over the partition
axis, so both operands are fed with the contraction dim (d = 32768) on
partitions: the host pre-transposes each x shard to xT [32768, 1024] and
W to wT [32768, 768]. b_pre is folded into x on the host (x - b_pre)
before the transpose; with the reference's b_pre == 0 this is a no-op.

Default per-core kernel (DESIGN="sbuf", MM_DTYPE="bfloat16", F8=16):
mixed-precision against the 2e-2 rel-err gate. The first 224 d-chunks
stream as bf16 in supers of 8x128 rows (1 cyc/col on the PE, 78.6
TF/s/core, half the HBM bytes of fp32); the last 32 d-chunks (12.5% of
K) run as fp8e4 DoubleRow matmuls (K=256 per matmul, 157 TF/s). W's
fp8 shard is host-prescaled by 128 into e4m3's normal range; the psum
drain folds 1/128 back via one DVE scalar_tensor_tensor. For each of 8
output row-chunks, matmuls accumulate into [128, 768] PSUM tiles (x
stationary, wT moving 512+256) and the DVE adds PSUM into the
SBUF-resident [1024, 768] output; per-chunk output DMAs issue right
after each final drain on alternating sync/scalar queues. x DMAs issue
from the SP HWDGE engine and W DMAs from ACT.

Measured (HW, core-0 NTFF): 650.5 us at 95.2% PE-matmul occupancy,
scale-relative error 1.41e-2 (deterministic; gate is 2e-2). MATMUL
busy 625 us = the PE floor at the sustained ~2.37 GHz clock. Pure bf16
(F8=0): 689 us at 2.5e-3. fp32r baseline: 826 us at 1.6e-4. Note:
runs occasionally land on a ~1.98 GHz pstate and read ~15% slower
across every engine (one in ~5 runs observed).

Tuning notes: fp8 beyond ~16 chunk-pairs erodes the error margin
(err scales as sqrt(f8_fraction); measured 1.34e-2 rms at F8=16);
full fp8 measures 3.9e-2 and fails. XT/WT_BUFS=16 starves prefetch at
pool-wrap (x stream lands just-in-time ~210 GB/s/queue); 24 is safe
with bf16 tile sizes. A [4,4]-chunk warmup super schedule did not
help. DoubleRow psum tiles are bank-charged 2x -> PS_BUFS=2 with F8.
"""

import os
import sys

if "/opt/trn_rl_repo" not in sys.path:
    sys.path.insert(0, "/opt/trn_rl_repo")

import numpy as np


def _shim_axon_hooks():
    """The agent image's `antenv` package lacks `axon_hooks`; when a
    caller sets BASS_TRACE=1 the bass trace path would ImportError. Shim
    the module (registering the same ctypes NTFF hook trn_boot would
    have) so tracing degrades gracefully instead of crashing."""
    try:
        import antenv.axon_hooks  # noqa: F401
        return
    except ImportError:
        pass
    try:
        import types

        import antenv
        from trn_agent_boot.trn_boot import _ntff_profile_via_ctypes

        m = types.ModuleType("antenv.axon_hooks")
        m._hook = _ntff_profile_via_ctypes("/opt/axon/libaxon_pjrt.so")
        m.get_axon_ntff_profile_hook = lambda: m._hook
        m.set_axon_ntff_profile_hook = lambda h: setattr(m, "_hook", h)
        sys.modules["antenv.axon_hooks"] = m
        antenv.axon_hooks = m
    except Exception:
        pass


_shim_axon_hooks()

N_TOK = 8192
D_IN = 32768
D_OUT = 768
N_CORES = 8
N_SHARD = N_TOK // N_CORES          # 1024 token rows per core
P = 128
D_CHUNKS = D_IN // P                # 256
N_SUPER = 512                       # token rows resident in PSUM at once
N_SUPERS = N_SHARD // N_SUPER       # 2
N_CH = N_SUPER // P                 # 4 psum tiles per n-block

# Matmul input dtype knob:
#   "bfloat16": 1 cyc/col on the PE (78.6 TF/s/core) and half the HBM
#       traffic of fp32. Scale-relative error ~1.5e-3 at K=32768 --
#       well under the 2e-2 gate.
#   "float32r": single-pass fp32 PE matmul, ~9/8 cyc/col measured, max
#       scale-relative error 1.6e-4.
#   "float32": exact to 1e-6 but 4 cyc/col -> ~3.5x slower.
MM_DTYPE = os.environ.get("KERNEL_MM_DTYPE", "bfloat16")

# Max acceptable scale-relative error (vs fp64, 64 sampled rows) before
# kernel() falls back to a more exact dtype.
_ERR_GATE = {"bfloat16": 8e-3, "float16": 8e-3, "float32r": 5e-3,
             "float32": 1e-4}
# "sbuf": d-super blocking, output accumulated in SBUF, min DMA traffic,
#         optional fp8 DoubleRow tail (best: 650 us HW).
# "dp_ot": W-stationary all-512-moving variant, output transposed
#          (measured equal to "sbuf" at pure bf16: 690 us).
# "psum": full-K accumulation in PSUM, W streamed twice (simplest).
# "kshard"/"kshard_ot": tensor-parallel over the contraction dim.
DESIGN = os.environ.get("KERNEL_DESIGN", "sbuf")

LAST_RESULTS = None  # BassKernelResults of the most recent kernel() call


W8_SCALE = 128.0  # host pre-scales the fp8 W shard into e4m3 normal range


def _f8_pairs() -> int:
    """Chunk-PAIRS (2x128 K rows each) computed in fp8 DoubleRow. Only
    meaningful for the bfloat16 fast path."""
    if MM_DTYPE != "bfloat16":
        return 0
    return _F8_STATE[0]


_F8_STATE = [int(os.environ.get("KERNEL_F8_PAIRS", "16"))]


def _build_bass_sbuf():
    """Design 1: stream xT and wT exactly once in d-supers; accumulate
    the [1024, 768] output in SBUF across d-supers (DVE adds PSUM into
    the resident C tiles).

    Refinements over the fp32r baseline:
    - optional fp8 tail: the last 2*F8 d-chunks are computed in
      DoubleRow fp8e4 (2x PE rate, K=256 per matmul). W is pre-scaled
      by W8_SCALE on the host; the drain folds 1/W8_SCALE back in via
      scalar_tensor_tensor (ct = ps*(1/s) + ct);
    - per-chunk output DMAs issued right after each chunk's last drain,
      alternating the sync/scalar issue queues, so the writeback
      overlaps the remaining compute."""
    import concourse.mybir as mybir
    import concourse.tile as tile
    from concourse import bacc

    dt_mm = getattr(mybir.dt, MM_DTYPE)
    f32 = mybir.dt.float32
    f8 = mybir.dt.float8e4
    F8 = _f8_pairs()
    D8 = 2 * F8 * P                # fp8 K rows (tail of the d axis)
    DB = D_IN - D8                 # bf16/fp32 K rows
    CHB = DB // P                  # full-precision d-chunks
    DSUP = int(os.environ.get("KERNEL_DSUP", "8"))  # d-chunks per super
    sched = [DSUP] * (CHB // DSUP)
    assert sum(sched) == CHB, (sched, CHB)
    NCH = N_SHARD // P             # 8 output row-chunks

    nc = bacc.Bacc(None, target_bir_lowering=False)
    xT = nc.dram_tensor("xT", [DB, N_SHARD], dt_mm, kind="ExternalInput")
    wT = nc.dram_tensor("wT", [DB, D_OUT], dt_mm, kind="ExternalInput")
    if F8:
        x8T = nc.dram_tensor("x8T", [D8, N_SHARD], f8, kind="ExternalInput")
        w8T = nc.dram_tensor("w8T", [D8, D_OUT], f8, kind="ExternalInput")
    out = nc.dram_tensor("out", [N_SHARD, D_OUT], f32, kind="ExternalOutput")

    XT_BUFS = int(os.environ.get("KERNEL_XT_BUFS", "24"))
    WT_BUFS = int(os.environ.get("KERNEL_WT_BUFS", "24"))
    # DoubleRow-mode psum tiles are charged 2x banks by the allocator,
    # so with the fp8 tail only 2 bufs fit (2x4 banks); still enough to
    # hide each drain under the next matmul group.
    PS_BUFS = int(os.environ.get("KERNEL_PS_BUFS", "2" if F8 else "3"))
    mul = mybir.AluOpType.mult
    add = mybir.AluOpType.add
    with tile.TileContext(nc) as tc:
        with (
            tc.tile_pool(name="xs", bufs=XT_BUFS) as xpool,
            tc.tile_pool(name="ws", bufs=WT_BUFS) as wpool,
            tc.tile_pool(name="c", bufs=1) as cpool,
            tc.tile_pool(name="f8x", bufs=max(F8, 1)) as f8xpool,
            tc.tile_pool(name="f8w", bufs=max(F8, 1)) as f8wpool,
            tc.tile_pool(name="psum", bufs=PS_BUFS, space="PSUM") as ppool,
        ):
            cts = [
                cpool.tile([P, D_OUT], f32, name=f"c{i}") for i in range(NCH)
            ]
            x8ts = []
            w8ts = []
            chunk0 = 0
            for si, DS in enumerate(sched):
                # Per-chunk tiles (not one slab) so the first matmul of a
                # super only waits on one chunk DMA, and prefetch runs
                # chunk-granular across supers.
                xts = []
                wts = []
                for j in range(DS):
                    row = (chunk0 + j) * P
                    xt = xpool.tile([P, N_SHARD], dt_mm, name="xt")
                    wt = wpool.tile([P, D_OUT], dt_mm, name="wt")
                    # Split descriptor issue across the two HWDGE engines
                    # (SP + ACT) so x and W prefetch don't queue behind
                    # each other on one issue path.
                    nc.sync.dma_start(xt[:], xT[row:row + P, :])
                    nc.scalar.dma_start(wt[:], wT[row:row + P, :])
                    xts.append(xt)
                    wts.append(wt)
                if si == len(sched) - 1 and F8:
                    # fp8 tail operands: issued after all bf16 prefetch
                    # DMAs so they never starve the main stream; they
                    # land well before the fp8 section runs.
                    for p8 in range(F8):
                        x8t = f8xpool.tile([P, 2, N_SHARD], f8, name="x8t")
                        w8t = f8wpool.tile([P, 2, D_OUT], f8, name="w8t")
                        for ks in range(2):
                            row = (2 * p8 + ks) * P
                            nc.sync.dma_start(x8t[:, ks, :],
                                              x8T[row:row + P, :])
                            nc.scalar.dma_start(w8t[:, ks, :],
                                                w8T[row:row + P, :])
                        x8ts.append(x8t)
                        w8ts.append(w8t)
                for nch in range(NCH):
                    ps = ppool.tile([P, D_OUT], f32, name="ps")
                    for j in range(DS):
                        lhsT = xts[j][:, nch * P:(nch + 1) * P]
                        nc.tensor.matmul(
                            ps[:, 0:512], lhsT, wts[j][:, 0:512],
                            start=(j == 0), stop=(j == DS - 1),
                        )
                        nc.tensor.matmul(
                            ps[:, 512:D_OUT], lhsT, wts[j][:, 512:D_OUT],
                            start=(j == 0), stop=(j == DS - 1),
                        )
                    if si == 0:
                        nc.vector.tensor_copy(cts[nch][:], ps[:])
                    else:
                        nc.vector.tensor_add(cts[nch][:], cts[nch][:], ps[:])
                    if si == len(sched) - 1 and not F8:
                        eng = nc.sync if nch % 2 == 0 else nc.scalar
                        eng.dma_start(out[nch * P:(nch + 1) * P, :],
                                      cts[nch][:])
                chunk0 += DS
            if F8:
                dr = mybir.MatmulPerfMode.DoubleRow
                for nch in range(NCH):
                    ps = ppool.tile([P, D_OUT], f32, name="ps8")
                    for p8 in range(F8):
                        lhsT = x8ts[p8][:, :, nch * P:(nch + 1) * P]
                        nc.tensor.matmul(
                            ps[:, 0:512], lhsT, w8ts[p8][:, :, 0:512],
                            start=(p8 == 0), stop=(p8 == F8 - 1),
                            perf_mode=dr,
                        )
                        nc.tensor.matmul(
                            ps[:, 512:D_OUT], lhsT,
                            w8ts[p8][:, :, 512:D_OUT],
                            start=(p8 == 0), stop=(p8 == F8 - 1),
                            perf_mode=dr,
                        )
                    nc.vector.scalar_tensor_tensor(
                        cts[nch][:], ps[:], 1.0 / W8_SCALE, cts[nch][:],
                        op0=mul, op1=add,
                    )
                    eng = nc.sync if nch % 2 == 0 else nc.scalar
                    eng.dma_start(out[nch * P:(nch + 1) * P, :], cts[nch][:])

    nc.compile()
    return nc


def _build_bass_dp_ot():
    """Design 5 (data-parallel, W-stationary, all-512 moving): like
    "sbuf" but with wT chunks as the stationary operand and xT as the
    512-wide moving operand, so every matmul streams 512 cols (213 ns at
    bf16) and fully hides its ~97 ns LDWEIGHTS (the x-stationary 256-col
    matmuls at 107 ns barely cover it). Output accumulates in SBUF
    transposed [768, N_SHARD]; the host transposes back at gather. Final
    out DMAs alternate sync/scalar queues and are issued per-chunk so the
    drain overlaps the last super's adds."""
    import concourse.mybir as mybir
    import concourse.tile as tile
    from concourse import bacc

    dt_mm = getattr(mybir.dt, MM_DTYPE)
    f32 = mybir.dt.float32
    DSUP = int(os.environ.get("KERNEL_DSUP", "8"))  # d-chunks per super
    NSUP = D_CHUNKS // DSUP        # supers
    OC = D_OUT // P                # 6 output-channel chunks
    TH = N_SHARD // 512            # 2 moving halves of 512 tokens

    nc = bacc.Bacc(None, target_bir_lowering=False)
    xT = nc.dram_tensor("xT", [D_IN, N_SHARD], dt_mm, kind="ExternalInput")
    wT = nc.dram_tensor("wT", [D_IN, D_OUT], dt_mm, kind="ExternalInput")
    outT = nc.dram_tensor("outT", [D_OUT, N_SHARD], f32, kind="ExternalOutput")

    XT_BUFS = int(os.environ.get("KERNEL_XT_BUFS", "16"))
    WT_BUFS = int(os.environ.get("KERNEL_WT_BUFS", "16"))
    PS_BUFS = int(os.environ.get("KERNEL_PS_BUFS", "3"))
    with tile.TileContext(nc) as tc:
        with (
            tc.tile_pool(name="xs", bufs=XT_BUFS) as xpool,
            tc.tile_pool(name="ws", bufs=WT_BUFS) as wpool,
            tc.tile_pool(name="c", bufs=1) as cpool,
            tc.tile_pool(name="psum", bufs=PS_BUFS, space="PSUM") as ppool,
        ):
            cts = [
                cpool.tile([P, N_SHARD], f32, name=f"c{i}") for i in range(OC)
            ]
            for ds in range(NSUP):
                xts = []
                wts = []
                for j in range(DSUP):
                    row = (ds * DSUP + j) * P
                    xt = xpool.tile([P, N_SHARD], dt_mm, name="xt")
                    wt = wpool.tile([P, D_OUT], dt_mm, name="wt")
                    nc.sync.dma_start(xt[:], xT[row:row + P, :])
                    nc.scalar.dma_start(wt[:], wT[row:row + P, :])
                    xts.append(xt)
                    wts.append(wt)
                for oc in range(OC):
                    for th in range(TH):
                        ps = ppool.tile([P, 512], f32, name="ps")
                        for j in range(DSUP):
                            nc.tensor.matmul(
                                ps[:],
                                wts[j][:, oc * P:(oc + 1) * P],
                                xts[j][:, th * 512:(th + 1) * 512],
                                start=(j == 0), stop=(j == DSUP - 1),
                            )
                        dst = cts[oc][:, th * 512:(th + 1) * 512]
                        if ds == 0:
                            nc.vector.tensor_copy(dst, ps[:])
                        else:
                            nc.vector.tensor_add(dst, dst, ps[:])
                    if ds == NSUP - 1:
                        eng = nc.sync if oc % 2 == 0 else nc.scalar
                        eng.dma_start(outT[oc * P:(oc + 1) * P, :], cts[oc][:])

    nc.compile()
    return nc


def _build_bass_kshard():
    """Design 3 (tensor-parallel): shard the contraction dim d across
    cores (4096 rows each). The W^T shard [4096, 768] (12 MiB) stays
    resident in SBUF; x^T [4096, 8192] streams through once. Each core
    produces a full [8192, 768] partial; the host reduces the 8 partials
    at gather time (the sharding hint's "all-reduce on the [N,768]
    output"). PSUM accumulates the core's entire local contraction."""
    import concourse.mybir as mybir
    import concourse.tile as tile
    from concourse import bacc

    dt_mm = getattr(mybir.dt, MM_DTYPE)
    f32 = mybir.dt.float32
    D_SHARD = D_IN // N_CORES       # 4096 contraction rows per core
    DC = D_SHARD // P               # 32 d-chunks
    NB = N_TOK // N_SUPER           # 16 n-blocks of 512 token rows

    nc = bacc.Bacc(None, target_bir_lowering=False)
    xT = nc.dram_tensor("xT", [D_SHARD, N_TOK], dt_mm, kind="ExternalInput")
    wT = nc.dram_tensor("wT", [D_SHARD, D_OUT], dt_mm, kind="ExternalInput")
    out = nc.dram_tensor("out", [N_TOK, D_OUT], f32, kind="ExternalOutput")

    with tile.TileContext(nc) as tc:
        with (
            tc.tile_pool(name="w", bufs=1) as wpool,
            tc.tile_pool(name="xt", bufs=4) as xpool,
            tc.tile_pool(name="ot", bufs=4) as opool,
            tc.tile_pool(name="psum", bufs=1, space="PSUM") as ppool,
        ):
            ws = wpool.tile([P, DC, D_OUT], dt_mm, name="ws")
            for j in range(DC):
                nc.sync.dma_start(ws[:, j, :], wT[j * P:(j + 1) * P, :])
            for nb in range(NB):
                psums = [
                    ppool.tile([P, D_OUT], f32, name=f"psum{i}")
                    for i in range(N_CH)
                ]
                for dc in range(DC):
                    xt = xpool.tile([P, N_SUPER], dt_mm, name="xt")
                    nc.sync.dma_start(
                        xt[:],
                        xT[dc * P:(dc + 1) * P,
                           nb * N_SUPER:(nb + 1) * N_SUPER],
                    )
                    st = dc == 0
                    sp = dc == DC - 1
                    for nch in range(N_CH):
                        lhsT = xt[:, nch * P:(nch + 1) * P]
                        nc.tensor.matmul(
                            psums[nch][:, 0:512], lhsT, ws[:, dc, 0:512],
                            start=st, stop=sp,
                        )
                        nc.tensor.matmul(
                            psums[nch][:, 512:D_OUT], lhsT,
                            ws[:, dc, 512:D_OUT],
                            start=st, stop=sp,
                        )
                for nch in range(N_CH):
                    ot = opool.tile([P, D_OUT], f32, name="ot")
                    nc.vector.tensor_copy(ot[:], psums[nch][:])
                    base = nb * N_SUPER + nch * P
                    nc.sync.dma_start(out[base:base + P, :], ot[:])

    nc.compile()
    return nc


def _build_bass_kshard_ot():
    """Design 4 (tensor-parallel, W-stationary): like kshard, but W^T
    tiles are the stationary operand and x^T streams as the moving side,
    so every matmul has a 512-wide moving operand. For float32r each
    matmul self-loads its stationary via a ~214 ns LDWEIGHTS; with all
    matmuls at N=512 (213 ns) the loads pipeline behind the previous
    matmul instead of stalling (the N=256 matmuls of the x-stationary
    designs are LDW-bound). Output lands transposed [768, 8192]; the
    host transposes back during the reduce."""
    import concourse.mybir as mybir
    import concourse.tile as tile
    from concourse import bacc

    dt_mm = getattr(mybir.dt, MM_DTYPE)
    f32 = mybir.dt.float32
    D_SHARD = D_IN // N_CORES       # 4096 contraction rows per core
    DC = D_SHARD // P               # 32 d-chunks
    NB = N_TOK // 512               # 16 moving n-blocks
    OC = D_OUT // P                 # 6 output-channel chunks

    nc = bacc.Bacc(None, target_bir_lowering=False)
    xT = nc.dram_tensor("xT", [D_SHARD, N_TOK], dt_mm, kind="ExternalInput")
    wT = nc.dram_tensor("wT", [D_SHARD, D_OUT], dt_mm, kind="ExternalInput")
    outT = nc.dram_tensor("outT", [D_OUT, N_TOK], f32, kind="ExternalOutput")

    with tile.TileContext(nc) as tc:
        with (
            tc.tile_pool(name="w", bufs=1) as wpool,
            tc.tile_pool(name="xt", bufs=4) as xpool,
            tc.tile_pool(name="ot", bufs=4) as opool,
            tc.tile_pool(name="psum", bufs=1, space="PSUM") as ppool,
        ):
            ws = wpool.tile([P, DC, D_OUT], dt_mm, name="ws")
            for j in range(DC):
                nc.sync.dma_start(ws[:, j, :], wT[j * P:(j + 1) * P, :])
            for nb in range(NB):
                psums = [
                    ppool.tile([P, 512], f32, name=f"psum{i}")
                    for i in range(OC)
                ]
                for dc in range(DC):
                    xt = xpool.tile([P, 512], dt_mm, name="xt")
                    nc.sync.dma_start(
                        xt[:], xT[dc * P:(dc + 1) * P, nb * 512:(nb + 1) * 512]
                    )
                    st = dc == 0
                    sp = dc == DC - 1
                    for oc in range(OC):
                        nc.tensor.matmul(
                            psums[oc][:], ws[:, dc, oc * P:(oc + 1) * P],
                            xt[:], start=st, stop=sp,
                        )
                for oc in range(OC):
                    ot = opool.tile([P, 512], f32, name="ot")
                    nc.vector.tensor_copy(ot[:], psums[oc][:])
                    nc.sync.dma_start(
                        outT[oc * P:(oc + 1) * P, nb * 512:(nb + 1) * 512],
                        ot[:],
                    )

    nc.compile()
    return nc


def _build_bass():
    if DESIGN == "sbuf":
        return _build_bass_sbuf()
    if DESIGN == "dp_ot":
        return _build_bass_dp_ot()
    if DESIGN == "kshard":
        return _build_bass_kshard()
    if DESIGN == "kshard_ot":
        return _build_bass_kshard_ot()
    import concourse.mybir as mybir
    import concourse.tile as tile
    from concourse import bacc

    dt_mm = getattr(mybir.dt, MM_DTYPE)
    f32 = mybir.dt.float32

    nc = bacc.Bacc(None, target_bir_lowering=False)
    xT = nc.dram_tensor("xT", [D_IN, N_SHARD], dt_mm, kind="ExternalInput")
    wT = nc.dram_tensor("wT", [D_IN, D_OUT], dt_mm, kind="ExternalInput")
    out = nc.dram_tensor("out", [N_SHARD, D_OUT], f32, kind="ExternalOutput")

    with tile.TileContext(nc) as tc:
        with (
            tc.tile_pool(name="xt", bufs=4) as xpool,
            tc.tile_pool(name="wt", bufs=4) as wpool,
            tc.tile_pool(name="ot", bufs=4) as opool,
            tc.tile_pool(name="psum", bufs=1, space="PSUM") as ppool,
        ):
            for ns in range(N_SUPERS):
                psums = [
                    ppool.tile([P, D_OUT], f32, name=f"psum{i}")
                    for i in range(N_CH)
                ]
                for dc in range(D_CHUNKS):
                    xt = xpool.tile([P, N_SUPER], dt_mm)
                    wt = wpool.tile([P, D_OUT], dt_mm)
                    nc.sync.dma_start(
                        xt[:],
                        xT[dc * P:(dc + 1) * P, ns * N_SUPER:(ns + 1) * N_SUPER],
                    )
                    nc.sync.dma_start(wt[:], wT[dc * P:(dc + 1) * P, :])
                    st = dc == 0
                    sp = dc == D_CHUNKS - 1
                    for nch in range(N_CH):
                        lhsT = xt[:, nch * P:(nch + 1) * P]
                        nc.tensor.matmul(
                            psums[nch][:, 0:512], lhsT, wt[:, 0:512],
                            start=st, stop=sp,
                        )
                        nc.tensor.matmul(
                            psums[nch][:, 512:D_OUT], lhsT, wt[:, 512:D_OUT],
                            start=st, stop=sp,
                        )
                for nch in range(N_CH):
                    ot = opool.tile([P, D_OUT], f32)
                    nc.vector.tensor_copy(ot[:], psums[nch][:])
                    base = ns * N_SUPER + nch * P
                    nc.sync.dma_start(out[base:base + P, :], ot[:])

    nc.compile()
    return nc


def kernel(x: np.ndarray, W: np.ndarray, b_pre: np.ndarray) -> np.ndarray:
    global MM_DTYPE

    x = np.asarray(x, dtype=np.float32)
    W = np.asarray(W, dtype=np.float32)
    b_pre = np.asarray(b_pre, dtype=np.float32)

    # Fold the pre-bias on the host (exact no-op for b_pre == 0).
    if b_pre.any():
        x = x - b_pre[None, :]

    out = _run_device(x, W)

    # Cheap sampled sanity check (64 rows vs numpy fp64). Expected
    # scale-relative error: ~1.3e-2 for bf16+fp8 hybrid (F8=16),
    # ~1.5e-3 for pure bfloat16, ~1.6e-4 for float32r. Anything above
    # the gate means the fast path misbehaved on this machine -> retry
    # one tier more exact (drop fp8 first, then drop bf16).
    idx = np.arange(0, N_TOK, N_TOK // 64)
    ref = x[idx].astype(np.float64) @ W.astype(np.float64).T
    err = np.abs(out[idx] - ref).max() / (np.abs(ref).max() + 1e-30)
    gate = 1.7e-2 if _f8_pairs() else _ERR_GATE.get(MM_DTYPE, 5e-3)
    if not np.isfinite(err) or err > gate:
        if _f8_pairs():
            _F8_STATE[0] = 0
            out = kernel(x, W, np.zeros_like(b_pre))
        elif MM_DTYPE != "float32":
            MM_DTYPE = "float32r" if MM_DTYPE in ("bfloat16", "float16") \
                else "float32"
            out = kernel(x, W, np.zeros_like(b_pre))
    return out


def _run_device(x: np.ndarray, W: np.ndarray) -> np.ndarray:
    global LAST_RESULTS
    from concourse.bass_utils import run_bass_kernel_spmd

    x8 = W8 = None
    F8 = _f8_pairs() if DESIGN == "sbuf" else 0
    if MM_DTYPE in ("bfloat16", "float16"):
        import ml_dtypes

        host_dt = np.dtype(getattr(ml_dtypes, MM_DTYPE))
        if F8:
            DB = D_IN - 2 * F8 * P
            f8dt = np.dtype(ml_dtypes.float8_e4m3)
            x8 = x[:, DB:].astype(f8dt)
            W8 = (W[:, DB:] * W8_SCALE).astype(f8dt)
            x = x[:, :DB].astype(host_dt)
            W = W[:, :DB].astype(host_dt)
        else:
            x = x.astype(host_dt)
            W = W.astype(host_dt)

    wTc = np.ascontiguousarray(W.T)  # [D_IN(-D8), D_OUT]
    if DESIGN in ("kshard", "kshard_ot"):
        D_SHARD = D_IN // N_CORES
        xTfull = np.ascontiguousarray(x.T)  # [D_IN, N_TOK]
        in_maps = [{
            "xT": xTfull[c * D_SHARD:(c + 1) * D_SHARD],
            "wT": wTc[c * D_SHARD:(c + 1) * D_SHARD],
        } for c in range(N_CORES)]
    else:
        in_maps = [{
            "xT": np.ascontiguousarray(x[c * N_SHARD:(c + 1) * N_SHARD].T),
            "wT": wTc,
        } for c in range(N_CORES)]
        if F8:
            w8Tc = np.ascontiguousarray(W8.T)
            for c in range(N_CORES):
                in_maps[c]["x8T"] = np.ascontiguousarray(
                    x8[c * N_SHARD:(c + 1) * N_SHARD].T
                )
                in_maps[c]["w8T"] = w8Tc

    nc = _build_bass()
    last_err = None
    for attempt in range(3):
        try:
            LAST_RESULTS = run_bass_kernel_spmd(
                nc, in_maps, core_ids=list(range(N_CORES)),
                tmpdir=os.environ.get("KERNEL_TRACE_DIR") or None,
            )
            break
        except Exception as e:  # transient device faults recover on retry
            last_err = e
            import time

            time.sleep(10)
    else:
        raise last_err
    if DESIGN == "kshard":
        # Tensor-parallel: reduce the per-core partials (host all-reduce).
        acc = np.zeros((N_TOK, D_OUT), dtype=np.float64)
        for c in range(N_CORES):
            acc += LAST_RESULTS.results[c]["out"]
        out = acc.astype(np.float32)
    elif DESIGN == "kshard_ot":
        acc = np.zeros((D_OUT, N_TOK), dtype=np.float64)
        for c in range(N_CORES):
            acc += LAST_RESULTS.results[c]["outT"]
        out = np.ascontiguousarray(acc.T.astype(np.float32))
    elif DESIGN == "dp_ot":
        out = np.concatenate(
            [
                np.ascontiguousarray(LAST_RESULTS.results[c]["outT"].T)
                for c in range(N_CORES)
            ],
            axis=0,
        )
    else:
        out = np.concatenate(
            [LAST_RESULTS.results[c]["out"] for c in range(N_CORES)], axis=0
        )
    return out



# revision 23
# speedup vs baseline: 1.2320x; 1.0264x over previous
"""Trainium2 Bass kernel for nn_Decoder: out = (x - b_pre) @ W^T.

Shapes (hardcoded): x [8192, 32768] f32, W [768, 32768] f32, b_pre [32768] f32
-> out [8192, 768] f32.

Sharding: data-parallel over the 8192 token rows across 8 NeuronCores
(1024 rows each), W replicated. The TensorE contracts over the partition
axis, so both operands are fed with the contraction dim (d = 32768) on
partitions: the host pre-transposes each x shard to xT [32768, 1024] and
W to wT [32768, 768]. b_pre is folded into x on the host (x - b_pre)
before the transpose; with the reference's b_pre == 0 this is a no-op.

Default per-core kernel (DESIGN="sbuf", MM_DTYPE="bfloat16", F8=20):
mixed-precision against the 2e-2 rel-err gate. The first 216 d-chunks
stream as bf16 in supers of 8x128 rows (1 cyc/col on the PE, 78.6
TF/s/core, half the HBM bytes of fp32); the last 40 d-chunks (15.6% of
K) run as fp8e4 DoubleRow matmuls (K=256 per matmul, 157 TF/s). W's
fp8 shard is host-prescaled by 128 into e4m3's normal range; the psum
drain folds 1/128 back via one DVE scalar_tensor_tensor. For each of 8
output row-chunks, matmuls accumulate into [128, 768] PSUM tiles (x
stationary, wT moving 512+256) and the DVE adds PSUM into the
SBUF-resident [1024, 768] output; per-chunk output DMAs issue right
after each final drain on alternating sync/scalar queues. x DMAs issue
from the SP HWDGE engine and W DMAs from ACT.

Measured (HW, core-0 NTFF): 646.4 us, scale-relative error 1.547e-2
(bit-deterministic across runs; gate is 2e-2, margin 1.29x). MATMUL
busy 614.7 us = the PE floor at the sustained ~2.37 GHz clock. F8=16:
650.5 us at 1.408e-2. Pure bf16 (F8=0): 689 us at 2.5e-3. fp32r
baseline: 826 us at 1.6e-4. Note: runs occasionally land on a
~1.98 GHz pstate and read ~15% slower across every engine (one in ~6
runs observed).

Tuning notes: fp8 error scales as sqrt(f8_fraction) of the measured
full-fp8 3.9e-2 (which fails the gate); F8=24 would measure ~1.7e-2 --
too close. XT/WT_BUFS=16 starves prefetch at pool-wrap (x stream lands
just-in-time ~210 GB/s/queue); 24 is safe with bf16 tile sizes. A
[4,4]-chunk warmup super schedule did not help. DoubleRow psum tiles
are bank-charged 2x -> PS_BUFS=2 with F8.
"""

import os
import sys

if "/opt/trn_rl_repo" not in sys.path:
    sys.path.insert(0, "/opt/trn_rl_repo")

import numpy as np


def _shim_axon_hooks():
    """The agent image's `antenv` package lacks `axon_hooks`; when a
    caller sets BASS_TRACE=1 the bass trace path would ImportError. Shim
    the module (registering the same ctypes NTFF hook trn_boot would
    have) so tracing degrades gracefully instead of crashing."""
    try:
        import antenv.axon_hooks  # noqa: F401
        return
    except ImportError:
        pass
    try:
        import types

        import antenv
        from trn_agent_boot.trn_boot import _ntff_profile_via_ctypes

        m = types.ModuleType("antenv.axon_hooks")
        m._hook = _ntff_profile_via_ctypes("/opt/axon/libaxon_pjrt.so")
        m.get_axon_ntff_profile_hook = lambda: m._hook
        m.set_axon_ntff_profile_hook = lambda h: setattr(m, "_hook", h)
        sys.modules["antenv.axon_hooks"] = m
        antenv.axon_hooks = m
    except Exception:
        pass


_shim_axon_hooks()

N_TOK = 8192
D_IN = 32768
D_OUT = 768
N_CORES = 8
N_SHARD = N_TOK // N_CORES          # 1024 token rows per core
P = 128
D_CHUNKS = D_IN // P                # 256
N_SUPER = 512                       # token rows resident in PSUM at once
N_SUPERS = N_SHARD // N_SUPER       # 2
N_CH = N_SUPER // P                 # 4 psum tiles per n-block

# Matmul input dtype knob:
#   "bfloat16": 1 cyc/col on the PE (78.6 TF/s/core) and half the HBM
#       traffic of fp32. Scale-relative error ~1.5e-3 at K=32768 --
#       well under the 2e-2 gate.
#   "float32r": single-pass fp32 PE matmul, ~9/8 cyc/col measured, max
#       scale-relative error 1.6e-4.
#   "float32": exact to 1e-6 but 4 cyc/col -> ~3.5x slower.
MM_DTYPE = os.environ.get("KERNEL_MM_DTYPE", "bfloat16")

# Max acceptable scale-relative error (vs fp64, 64 sampled rows) before
# kernel() falls back to a more exact dtype.
_ERR_GATE = {"bfloat16": 8e-3, "float16": 8e-3, "float32r": 5e-3,
             "float32": 1e-4}
# "sbuf": d-super blocking, output accumulated in SBUF, min DMA traffic,
#         optional fp8 DoubleRow tail (best: 646 us HW).
# "dp_ot": W-stationary all-512-moving variant, output transposed
#          (measured equal to "sbuf" at pure bf16: 690 us).
# "psum": full-K accumulation in PSUM, W streamed twice (simplest).
# "kshard"/"kshard_ot": tensor-parallel over the contraction dim.
DESIGN = os.environ.get("KERNEL_DESIGN", "sbuf")

LAST_RESULTS = None  # BassKernelResults of the most recent kernel() call


W8_SCALE = 128.0  # host pre-scales the fp8 W shard into e4m3 normal range


def _f8_pairs() -> int:
    """Chunk-PAIRS (2x128 K rows each) computed in fp8 DoubleRow. Only
    meaningful for the bfloat16 fast path."""
    if MM_DTYPE != "bfloat16":
        return 0
    return _F8_STATE[0]


_F8_STATE = [int(os.environ.get("KERNEL_F8_PAIRS", "20"))]


def _build_bass_sbuf():
    """Design 1: stream xT and wT exactly once in d-supers; accumulate
    the [1024, 768] output in SBUF across d-supers (DVE adds PSUM into
    the resident C tiles).

    Refinements over the fp32r baseline:
    - optional fp8 tail: the last 2*F8 d-chunks are computed in
      DoubleRow fp8e4 (2x PE rate, K=256 per matmul). W is pre-scaled
      by W8_SCALE on the host; the drain folds 1/W8_SCALE back in via
      scalar_tensor_tensor (ct = ps*(1/s) + ct);
    - per-chunk output DMAs issued right after each chunk's last drain,
      alternating the sync/scalar issue queues, so the writeback
      overlaps the remaining compute."""
    import concourse.mybir as mybir
    import concourse.tile as tile
    from concourse import bacc

    dt_mm = getattr(mybir.dt, MM_DTYPE)
    f32 = mybir.dt.float32
    f8 = mybir.dt.float8e4
    F8 = _f8_pairs()
    D8 = 2 * F8 * P                # fp8 K rows (tail of the d axis)
    DB = D_IN - D8                 # bf16/fp32 K rows
    CHB = DB // P                  # full-precision d-chunks
    DSUP = int(os.environ.get("KERNEL_DSUP", "8"))  # d-chunks per super
    sched = [DSUP] * (CHB // DSUP)
    assert sum(sched) == CHB, (sched, CHB)
    NCH = N_SHARD // P             # 8 output row-chunks

    nc = bacc.Bacc(None, target_bir_lowering=False)
    xT = nc.dram_tensor("xT", [DB, N_SHARD], dt_mm, kind="ExternalInput")
    wT = nc.dram_tensor("wT", [DB, D_OUT], dt_mm, kind="ExternalInput")
    if F8:
        x8T = nc.dram_tensor("x8T", [D8, N_SHARD], f8, kind="ExternalInput")
        w8T = nc.dram_tensor("w8T", [D8, D_OUT], f8, kind="ExternalInput")
    out = nc.dram_tensor("out", [N_SHARD, D_OUT], f32, kind="ExternalOutput")

    XT_BUFS = int(os.environ.get("KERNEL_XT_BUFS", "24"))
    WT_BUFS = int(os.environ.get("KERNEL_WT_BUFS", "24"))
    # DoubleRow-mode psum tiles are charged 2x banks by the allocator,
    # so with the fp8 tail only 2 bufs fit (2x4 banks); still enough to
    # hide each drain under the next matmul group.
    PS_BUFS = int(os.environ.get("KERNEL_PS_BUFS", "2" if F8 else "3"))
    mul = mybir.AluOpType.mult
    add = mybir.AluOpType.add
    with tile.TileContext(nc) as tc:
        with (
            tc.tile_pool(name="xs", bufs=XT_BUFS) as xpool,
            tc.tile_pool(name="ws", bufs=WT_BUFS) as wpool,
            tc.tile_pool(name="c", bufs=1) as cpool,
            tc.tile_pool(name="f8x", bufs=max(F8, 1)) as f8xpool,
            tc.tile_pool(name="f8w", bufs=max(F8, 1)) as f8wpool,
            tc.tile_pool(name="psum", bufs=PS_BUFS, space="PSUM") as ppool,
        ):
            cts = [
                cpool.tile([P, D_OUT], f32, name=f"c{i}") for i in range(NCH)
            ]
            x8ts = []
            w8ts = []
            chunk0 = 0
            for si, DS in enumerate(sched):
                # Per-chunk tiles (not one slab) so the first matmul of a
                # super only waits on one chunk DMA, and prefetch runs
                # chunk-granular across supers.
                xts = []
                wts = []
                for j in range(DS):
                    row = (chunk0 + j) * P
                    xt = xpool.tile([P, N_SHARD], dt_mm, name="xt")
                    wt = wpool.tile([P, D_OUT], dt_mm, name="wt")
                    # Split descriptor issue across the two HWDGE engines
                    # (SP + ACT) so x and W prefetch don't queue behind
                    # each other on one issue path.
                    nc.sync.dma_start(xt[:], xT[row:row + P, :])
                    nc.scalar.dma_start(wt[:], wT[row:row + P, :])
                    xts.append(xt)
                    wts.append(wt)
                if si == len(sched) - 1 and F8:
                    # fp8 tail operands: issued after all bf16 prefetch
                    # DMAs so they never starve the main stream; they
                    # land well before the fp8 section runs.
                    for p8 in range(F8):
                        x8t = f8xpool.tile([P, 2, N_SHARD], f8, name="x8t")
                        w8t = f8wpool.tile([P, 2, D_OUT], f8, name="w8t")
                        for ks in range(2):
                            row = (2 * p8 + ks) * P
                            nc.sync.dma_start(x8t[:, ks, :],
                                              x8T[row:row + P, :])
                            nc.scalar.dma_start(w8t[:, ks, :],
                                                w8T[row:row + P, :])
                        x8ts.append(x8t)
                        w8ts.append(w8t)
                for nch in range(NCH):
                    ps = ppool.tile([P, D_OUT], f32, name="ps")
                    for j in range(DS):
                        lhsT = xts[j][:, nch * P:(nch + 1) * P]
                        nc.tensor.matmul(
                            ps[:, 0:512], lhsT, wts[j][:, 0:512],
                            start=(j == 0), stop=(j == DS - 1),
                        )
                        nc.tensor.matmul(
                            ps[:, 512:D_OUT], lhsT, wts[j][:, 512:D_OUT],
                            start=(j == 0), stop=(j == DS - 1),
                        )
                    if si == 0:
                        nc.vector.tensor_copy(cts[nch][:], ps[:])
                    else:
                        nc.vector.tensor_add(cts[nch][:], cts[nch][:], ps[:])
                    if si == len(sched) - 1 and not F8:
                        eng = nc.sync if nch % 2 == 0 else nc.scalar
                        eng.dma_start(out[nch * P:(nch + 1) * P, :],
                                      cts[nch][:])
                chunk0 += DS
            if F8:
                dr = mybir.MatmulPerfMode.DoubleRow
                for nch in range(NCH):
                    ps = ppool.tile([P, D_OUT], f32, name="ps8")
                    for p8 in range(F8):
                        lhsT = x8ts[p8][:, :, nch * P:(nch + 1) * P]
                        nc.tensor.matmul(
                            ps[:, 0:512], lhsT, w8ts[p8][:, :, 0:512],
                            start=(p8 == 0), stop=(p8 == F8 - 1),
                            perf_mode=dr,
                        )
                        nc.tensor.matmul(
                            ps[:, 512:D_OUT], lhsT,
                            w8ts[p8][:, :, 512:D_OUT],
                            start=(p8 == 0), stop=(p8 == F8 - 1),
                            perf_mode=dr,
                        )
                    nc.vector.scalar_tensor_tensor(
                        cts[nch][:], ps[:], 1.0 / W8_SCALE, cts[nch][:],
                        op0=mul, op1=add,
                    )
                    eng = nc.sync if nch % 2 == 0 else nc.scalar
                    eng.dma_start(out[nch * P:(nch + 1) * P, :], cts[nch][:])

    nc.compile()
    return nc


def _build_bass_dp_ot():
    """Design 5 (data-parallel, W-stationary, all-512 moving): like
    "sbuf" but with wT chunks as the stationary operand and xT as the
    512-wide moving operand, so every matmul streams 512 cols (213 ns at
    bf16) and fully hides its ~97 ns LDWEIGHTS (the x-stationary 256-col
    matmuls at 107 ns barely cover it). Output accumulates in SBUF
    transposed [768, N_SHARD]; the host transposes back at gather. Final
    out DMAs alternate sync/scalar queues and are issued per-chunk so the
    drain overlaps the last super's adds."""
    import concourse.mybir as mybir
    import concourse.tile as tile
    from concourse import bacc

    dt_mm = getattr(mybir.dt, MM_DTYPE)
    f32 = mybir.dt.float32
    DSUP = int(os.environ.get("KERNEL_DSUP", "8"))  # d-chunks per super
    NSUP = D_CHUNKS // DSUP        # supers
    OC = D_OUT // P                # 6 output-channel chunks
    TH = N_SHARD // 512            # 2 moving halves of 512 tokens

    nc = bacc.Bacc(None, target_bir_lowering=False)
    xT = nc.dram_tensor("xT", [D_IN, N_SHARD], dt_mm, kind="ExternalInput")
    wT = nc.dram_tensor("wT", [D_IN, D_OUT], dt_mm, kind="ExternalInput")
    outT = nc.dram_tensor("outT", [D_OUT, N_SHARD], f32, kind="ExternalOutput")

    XT_BUFS = int(os.environ.get("KERNEL_XT_BUFS", "16"))
    WT_BUFS = int(os.environ.get("KERNEL_WT_BUFS", "16"))
    PS_BUFS = int(os.environ.get("KERNEL_PS_BUFS", "3"))
    with tile.TileContext(nc) as tc:
        with (
            tc.tile_pool(name="xs", bufs=XT_BUFS) as xpool,
            tc.tile_pool(name="ws", bufs=WT_BUFS) as wpool,
            tc.tile_pool(name="c", bufs=1) as cpool,
            tc.tile_pool(name="psum", bufs=PS_BUFS, space="PSUM") as ppool,
        ):
            cts = [
                cpool.tile([P, N_SHARD], f32, name=f"c{i}") for i in range(OC)
            ]
            for ds in range(NSUP):
                xts = []
                wts = []
                for j in range(DSUP):
                    row = (ds * DSUP + j) * P
                    xt = xpool.tile([P, N_SHARD], dt_mm, name="xt")
                    wt = wpool.tile([P, D_OUT], dt_mm, name="wt")
                    nc.sync.dma_start(xt[:], xT[row:row + P, :])
                    nc.scalar.dma_start(wt[:], wT[row:row + P, :])
                    xts.append(xt)
                    wts.append(wt)
                for oc in range(OC):
                    for th in range(TH):
                        ps = ppool.tile([P, 512], f32, name="ps")
                        for j in range(DSUP):
                            nc.tensor.matmul(
                                ps[:],
                                wts[j][:, oc * P:(oc + 1) * P],
                                xts[j][:, th * 512:(th + 1) * 512],
                                start=(j == 0), stop=(j == DSUP - 1),
                            )
                        dst = cts[oc][:, th * 512:(th + 1) * 512]
                        if ds == 0:
                            nc.vector.tensor_copy(dst, ps[:])
                        else:
                            nc.vector.tensor_add(dst, dst, ps[:])
                    if ds == NSUP - 1:
                        eng = nc.sync if oc % 2 == 0 else nc.scalar
                        eng.dma_start(outT[oc * P:(oc + 1) * P, :], cts[oc][:])

    nc.compile()
    return nc


def _build_bass_kshard():
    """Design 3 (tensor-parallel): shard the contraction dim d across
    cores (4096 rows each). The W^T shard [4096, 768] (12 MiB) stays
    resident in SBUF; x^T [4096, 8192] streams through once. Each core
    produces a full [8192, 768] partial; the host reduces the 8 partials
    at gather time (the sharding hint's "all-reduce on the [N,768]
    output"). PSUM accumulates the core's entire local contraction."""
    import concourse.mybir as mybir
    import concourse.tile as tile
    from concourse import bacc

    dt_mm = getattr(mybir.dt, MM_DTYPE)
    f32 = mybir.dt.float32
    D_SHARD = D_IN // N_CORES       # 4096 contraction rows per core
    DC = D_SHARD // P               # 32 d-chunks
    NB = N_TOK // N_SUPER           # 16 n-blocks of 512 token rows

    nc = bacc.Bacc(None, target_bir_lowering=False)
    xT = nc.dram_tensor("xT", [D_SHARD, N_TOK], dt_mm, kind="ExternalInput")
    wT = nc.dram_tensor("wT", [D_SHARD, D_OUT], dt_mm, kind="ExternalInput")
    out = nc.dram_tensor("out", [N_TOK, D_OUT], f32, kind="ExternalOutput")

    with tile.TileContext(nc) as tc:
        with (
            tc.tile_pool(name="w", bufs=1) as wpool,
            tc.tile_pool(name="xt", bufs=4) as xpool,
            tc.tile_pool(name="ot", bufs=4) as opool,
            tc.tile_pool(name="psum", bufs=1, space="PSUM") as ppool,
        ):
            ws = wpool.tile([P, DC, D_OUT], dt_mm, name="ws")
            for j in range(DC):
                nc.sync.dma_start(ws[:, j, :], wT[j * P:(j + 1) * P, :])
            for nb in range(NB):
                psums = [
                    ppool.tile([P, D_OUT], f32, name=f"psum{i}")
                    for i in range(N_CH)
                ]
                for dc in range(DC):
                    xt = xpool.tile([P, N_SUPER], dt_mm, name="xt")
                    nc.sync.dma_start(
                        xt[:],
                        xT[dc * P:(dc + 1) * P,
                           nb * N_SUPER:(nb + 1) * N_SUPER],
                    )
                    st = dc == 0
                    sp = dc == DC - 1
                    for nch in range(N_CH):
                        lhsT = xt[:, nch * P:(nch + 1) * P]
                        nc.tensor.matmul(
                            psums[nch][:, 0:512], lhsT, ws[:, dc, 0:512],
                            start=st, stop=sp,
                        )
                        nc.tensor.matmul(
                            psums[nch][:, 512:D_OUT], lhsT,
                            ws[:, dc, 512:D_OUT],
                            start=st, stop=sp,
                        )
                for nch in range(N_CH):
                    ot = opool.tile([P, D_OUT], f32, name="ot")
                    nc.vector.tensor_copy(ot[:], psums[nch][:])
                    base = nb * N_SUPER + nch * P
                    nc.sync.dma_start(out[base:base + P, :], ot[:])

    nc.compile()
    return nc


def _build_bass_kshard_ot():
    """Design 4 (tensor-parallel, W-stationary): like kshard, but W^T
    tiles are the stationary operand and x^T streams as the moving side,
    so every matmul has a 512-wide moving operand. For float32r each
    matmul self-loads its stationary via a ~214 ns LDWEIGHTS; with all
    matmuls at N=512 (213 ns) the loads pipeline behind the previous
    matmul instead of stalling (the N=256 matmuls of the x-stationary
    designs are LDW-bound). Output lands transposed [768, 8192]; the
    host transposes back during the reduce."""
    import concourse.mybir as mybir
    import concourse.tile as tile
    from concourse import bacc

    dt_mm = getattr(mybir.dt, MM_DTYPE)
    f32 = mybir.dt.float32
    D_SHARD = D_IN // N_CORES       # 4096 contraction rows per core
    DC = D_SHARD // P               # 32 d-chunks
    NB = N_TOK // 512               # 16 moving n-blocks
    OC = D_OUT // P                 # 6 output-channel chunks

    nc = bacc.Bacc(None, target_bir_lowering=False)
    xT = nc.dram_tensor("xT", [D_SHARD, N_TOK], dt_mm, kind="ExternalInput")
    wT = nc.dram_tensor("wT", [D_SHARD, D_OUT], dt_mm, kind="ExternalInput")
    outT = nc.dram_tensor("outT", [D_OUT, N_TOK], f32, kind="ExternalOutput")

    with tile.TileContext(nc) as tc:
        with (
            tc.tile_pool(name="w", bufs=1) as wpool,
            tc.tile_pool(name="xt", bufs=4) as xpool,
            tc.tile_pool(name="ot", bufs=4) as opool,
            tc.tile_pool(name="psum", bufs=1, space="PSUM") as ppool,
        ):
            ws = wpool.tile([P, DC, D_OUT], dt_mm, name="ws")
            for j in range(DC):
                nc.sync.dma_start(ws[:, j, :], wT[j * P:(j + 1) * P, :])
            for nb in range(NB):
                psums = [
                    ppool.tile([P, 512], f32, name=f"psum{i}")
                    for i in range(OC)
                ]
                for dc in range(DC):
                    xt = xpool.tile([P, 512], dt_mm, name="xt")
                    nc.sync.dma_start(
                        xt[:], xT[dc * P:(dc + 1) * P, nb * 512:(nb + 1) * 512]
                    )
                    st = dc == 0
                    sp = dc == DC - 1
                    for oc in range(OC):
                        nc.tensor.matmul(
                            psums[oc][:], ws[:, dc, oc * P:(oc + 1) * P],
                            xt[:], start=st, stop=sp,
                        )
                for oc in range(OC):
                    ot = opool.tile([P, 512], f32, name="ot")
                    nc.vector.tensor_copy(ot[:], psums[oc][:])
                    nc.sync.dma_start(
                        outT[oc * P:(oc + 1) * P, nb * 512:(nb + 1) * 512],
                        ot[:],
                    )

    nc.compile()
    return nc


def _build_bass():
    if DESIGN == "sbuf":
        return _build_bass_sbuf()
    if DESIGN == "dp_ot":
        return _build_bass_dp_ot()
    if DESIGN == "kshard":
        return _build_bass_kshard()
    if DESIGN == "kshard_ot":
        return _build_bass_kshard_ot()
    import concourse.mybir as mybir
    import concourse.tile as tile
    from concourse import bacc

    dt_mm = getattr(mybir.dt, MM_DTYPE)
    f32 = mybir.dt.float32

    nc = bacc.Bacc(None, target_bir_lowering=False)
    xT = nc.dram_tensor("xT", [D_IN, N_SHARD], dt_mm, kind="ExternalInput")
    wT = nc.dram_tensor("wT", [D_IN, D_OUT], dt_mm, kind="ExternalInput")
    out = nc.dram_tensor("out", [N_SHARD, D_OUT], f32, kind="ExternalOutput")

    with tile.TileContext(nc) as tc:
        with (
            tc.tile_pool(name="xt", bufs=4) as xpool,
            tc.tile_pool(name="wt", bufs=4) as wpool,
            tc.tile_pool(name="ot", bufs=4) as opool,
            tc.tile_pool(name="psum", bufs=1, space="PSUM") as ppool,
        ):
            for ns in range(N_SUPERS):
                psums = [
                    ppool.tile([P, D_OUT], f32, name=f"psum{i}")
                    for i in range(N_CH)
                ]
                for dc in range(D_CHUNKS):
                    xt = xpool.tile([P, N_SUPER], dt_mm)
                    wt = wpool.tile([P, D_OUT], dt_mm)
                    nc.sync.dma_start(
                        xt[:],
                        xT[dc * P:(dc + 1) * P, ns * N_SUPER:(ns + 1) * N_SUPER],
                    )
                    nc.sync.dma_start(wt[:], wT[dc * P:(dc + 1) * P, :])
                    st = dc == 0
                    sp = dc == D_CHUNKS - 1
                    for nch in range(N_CH):
                        lhsT = xt[:, nch * P:(nch + 1) * P]
                        nc.tensor.matmul(
                            psums[nch][:, 0:512], lhsT, wt[:, 0:512],
                            start=st, stop=sp,
                        )
                        nc.tensor.matmul(
                            psums[nch][:, 512:D_OUT], lhsT, wt[:, 512:D_OUT],
                            start=st, stop=sp,
                        )
                for nch in range(N_CH):
                    ot = opool.tile([P, D_OUT], f32)
                    nc.vector.tensor_copy(ot[:], psums[nch][:])
                    base = ns * N_SUPER + nch * P
                    nc.sync.dma_start(out[base:base + P, :], ot[:])

    nc.compile()
    return nc


def kernel(x: np.ndarray, W: np.ndarray, b_pre: np.ndarray) -> np.ndarray:
    global MM_DTYPE

    x = np.asarray(x, dtype=np.float32)
    W = np.asarray(W, dtype=np.float32)
    b_pre = np.asarray(b_pre, dtype=np.float32)

    # Fold the pre-bias on the host (exact no-op for b_pre == 0).
    if b_pre.any():
        x = x - b_pre[None, :]

    out = _run_device(x, W)

    # Cheap sampled sanity check (64 rows vs numpy fp64). Expected
    # scale-relative error: ~1.4e-2 for bf16+fp8 hybrid (F8=20),
    # ~1.5e-3 for pure bfloat16, ~1.6e-4 for float32r. Anything above
    # the gate means the fast path misbehaved on this machine -> retry
    # one tier more exact (drop fp8 first, then drop bf16).
    idx = np.arange(0, N_TOK, N_TOK // 64)
    ref = x[idx].astype(np.float64) @ W.astype(np.float64).T
    err = np.abs(out[idx] - ref).max() / (np.abs(ref).max() + 1e-30)
    gate = 1.7e-2 if _f8_pairs() else _ERR_GATE.get(MM_DTYPE, 5e-3)
    if not np.isfinite(err) or err > gate:
        if _f8_pairs():
            _F8_STATE[0] = 0
            out = kernel(x, W, np.zeros_like(b_pre))
        elif MM_DTYPE != "float32":
            MM_DTYPE = "float32r" if MM_DTYPE in ("bfloat16", "float16") \
                else "float32"
            out = kernel(x, W, np.zeros_like(b_pre))
    return out


def _run_device(x: np.ndarray, W: np.ndarray) -> np.ndarray:
    global LAST_RESULTS
    from concourse.bass_utils import run_bass_kernel_spmd

    x8 = W8 = None
    F8 = _f8_pairs() if DESIGN == "sbuf" else 0
    if MM_DTYPE in ("bfloat16", "float16"):
        import ml_dtypes

        host_dt = np.dtype(getattr(ml_dtypes, MM_DTYPE))
        if F8:
            DB = D_IN - 2 * F8 * P
            f8dt = np.dtype(ml_dtypes.float8_e4m3)
            x8 = x[:, DB:].astype(f8dt)
            W8 = (W[:, DB:] * W8_SCALE).astype(f8dt)
            x = x[:, :DB].astype(host_dt)
            W = W[:, :DB].astype(host_dt)
        else:
            x = x.astype(host_dt)
            W = W.astype(host_dt)

    wTc = np.ascontiguousarray(W.T)  # [D_IN(-D8), D_OUT]
    if DESIGN in ("kshard", "kshard_ot"):
        D_SHARD = D_IN // N_CORES
        xTfull = np.ascontiguousarray(x.T)  # [D_IN, N_TOK]
        in_maps = [{
            "xT": xTfull[c * D_SHARD:(c + 1) * D_SHARD],
            "wT": wTc[c * D_SHARD:(c + 1) * D_SHARD],
        } for c in range(N_CORES)]
    else:
        in_maps = [{
            "xT": np.ascontiguousarray(x[c * N_SHARD:(c + 1) * N_SHARD].T),
            "wT": wTc,
        } for c in range(N_CORES)]
        if F8:
            w8Tc = np.ascontiguousarray(W8.T)
            for c in range(N_CORES):
                in_maps[c]["x8T"] = np.ascontiguousarray(
                    x8[c * N_SHARD:(c + 1) * N_SHARD].T
                )
                in_maps[c]["w8T"] = w8Tc

    nc = _build_bass()
    last_err = None
    for attempt in range(3):
        try:
            LAST_RESULTS = run_bass_kernel_spmd(
                nc, in_maps, core_ids=list(range(N_CORES)),
                tmpdir=os.environ.get("KERNEL_TRACE_DIR") or None,
            )
            break
        except Exception as e:  # transient device faults recover on retry
            last_err = e
            import time

            time.sleep(10)
    else:
        raise last_err
    if DESIGN == "kshard":
        # Tensor-parallel: reduce the per-core partials (host all-reduce).
        acc = np.zeros((N_TOK, D_OUT), dtype=np.float64)
        for c in range(N_CORES):
            acc += LAST_RESULTS.results[c]["out"]
        out = acc.astype(np.float32)
    elif DESIGN == "kshard_ot":
        acc = np.zeros((D_OUT, N_TOK), dtype=np.float64)
        for c in range(N_CORES):
            acc += LAST_RESULTS.results[c]["outT"]
        out = np.ascontiguousarray(acc.T.astype(np.float32))
    elif DESIGN == "dp_ot":
        out = np.concatenate(
            [
                np.ascontiguousarray(LAST_RESULTS.results[c]["outT"].T)
                for c in range(N_CORES)
            ],
            axis=0,
        )
    else:
        out = np.concatenate(
            [LAST_RESULTS.results[c]["out"] for c in range(N_CORES)], axis=0
        )
    return out

